# revision 5
# baseline (speedup 1.0000x reference)
"""Trainium2 Bass kernel for nn_EndoSurfRenderer (EndoSurf-style renderer).

Contract: kernel(**inputs) takes the FULL inputs from setup_inputs() and
returns the FULL [65536, 8] output. Internally shards points across 8
NeuronCores (data parallel), replicates weights, runs one fused Bass/Tile
kernel per core, and gathers.

Pipeline per point-tile (feature-major on-chip layout [features, points]):
  deform MLP fwd + JVP tangent -> x_c, d_c ; sdf MLP fwd (softplus beta=100
  built from exp+ln) ; sdf VJP (sigmoid = 1-exp(-100*h')) -> gradients ;
  d_c normalize via exp(-0.5*ln(|d|^2)) ; color MLP ; assemble [N, 8].

All sin/cos computed as sin(2*pi*w) after exact range reduction
w = u - round(u), u = arg/(2*pi), round via the 1.5*2^23 magic constant.
All matmuls fp32 (fp32r is broken on this silicon; bf16 insufficient for
the beta=100 softplus sensitivity).
"""

import sys
from contextlib import ExitStack

if "/opt/trn_rl_repo" not in sys.path:
    sys.path.insert(0, "/opt/trn_rl_repo")

import numpy as np

import concourse.bacc as bacc
import concourse.tile as tile
from concourse import mybir
from concourse.bass_utils import run_bass_kernel_spmd

F32 = mybir.dt.float32
AF = mybir.ActivationFunctionType
ALU = mybir.AluOpType

NPTS = 65536
NCORES = 8
NPC = NPTS // NCORES        # 8192 points per core
P = 512                     # points per tile
NT = NPC // P               # tiles per core
MAGIC = float(1.5 * 2 ** 23)
TWOPI = float(2.0 * np.pi)
INV2PI = float(1.0 / (2.0 * np.pi))
SQRT2 = np.float32(np.sqrt(2.0))


# ===================== host-side packing (layout registry) ==================

class Pack:
    """Packs many [K, M] weight blocks into one [128, cols] f32 array.
    Each entry is placed at partition rows [base, base+K) and a column
    range; on-device it is read as an SBUF AP slice of one big tile."""

    def __init__(self):
        self.cols = 0
        self.entries = {}   # name -> (off, base, K, M)
        self.arrays = {}

    def add(self, name, arr, base=0):
        arr = np.ascontiguousarray(arr, np.float32)
        assert arr.ndim == 2
        K, M = arr.shape
        assert base + K <= 128, (name, base, K)
        self.entries[name] = (self.cols, base, K, M)
        self.arrays[name] = arr
        self.cols += M

    def materialize(self):
        w = np.zeros((128, self.cols), np.float32)
        for name, (off, base, K, M) in self.entries.items():
            w[base:base + K, off:off + M] = self.arrays[name]
        return w


def _trig_perm(D, L, base):
    sin_idx = [base + D + j * 2 * L + i for j in range(D) for i in range(L)]
    cos_idx = [base + D + j * 2 * L + L + i for j in range(D) for i in range(L)]
    return sin_idx, cos_idx


def build_packs(deform_params, sdf_params, color_params):
    """Returns (wpack, cpack) Pack objects with every weight/const block."""
    wp = Pack()
    cp = Pack()

    Wd = [np.asarray(w, np.float32) for (w, b) in deform_params]
    bd = [np.asarray(b, np.float32) for (w, b) in deform_params]
    Ws = [np.asarray(w, np.float32) for (w, b) in sdf_params]
    bs = [np.asarray(b, np.float32) for (w, b) in sdf_params]
    Wc = [np.asarray(w, np.float32) for (w, b) in color_params]
    bc = [np.asarray(b, np.float32) for (w, b) in color_params]

    # ---------------- deform enc helpers
    sx, cx = _trig_perm(3, 6, 0)
    st_, ct_ = _trig_perm(1, 6, 39)
    dtrig_rows = sx + st_ + cx + ct_          # 48: sin(x0..2,t) then cos(...)

    def split_dW0(W):
        trig = W[dtrig_rows, :]
        lin8 = np.zeros((8, W.shape[1]), np.float32)
        lin8[0:3] = W[0:3]
        lin8[6] = W[39]
        lint8 = np.zeros((8, W.shape[1]), np.float32)
        lint8[3:6] = W[0:3]
        return trig, lin8, lint8

    t0, l0, lt0 = split_dW0(Wd[0])
    l0[7] = bd[0]                             # bias via ones-row
    wp.add("d_W0_trig", t0)
    wp.add("d_W0_trig64", t0, base=64)        # dup for base-64 tangent rhs
    wp.add("d_W0_lin8", l0)
    wp.add("d_W0_lint8", lt0)
    for l in (1, 2, 3, 4, 6, 7):
        wp.add(f"d_W{l}_k0", Wd[l][0:128])
        wp.add(f"d_W{l}_k1", Wd[l][128:256])
    W5 = Wd[5] / SQRT2
    wp.add("d_W5_k0", W5[0:128])
    wp.add("d_W5_k1", W5[128:256])
    t5, l5, lt5 = split_dW0(W5[256:308])
    l5[7] = bd[5]
    wp.add("d_W5_trig", t5)
    wp.add("d_W5_trig64", t5, base=64)
    wp.add("d_W5_lin8", l5)
    wp.add("d_W5_lint8", lt5)
    wp.add("d_W8_k0", Wd[8][0:128])           # [128, 3]
    wp.add("d_W8_k1", Wd[8][128:256])

    # S matrices (args in u = arg/2pi space, ones-row carries +0.25 cos shift)
    S7u = np.zeros((8, 48), np.float32)
    S7Tu = np.zeros((8, 48), np.float32)
    Sd7 = np.zeros((8, 48), np.float32)
    for r in range(24):
        j, i = r // 6, r % 6
        row = 6 if j == 3 else j
        S7u[row, r] = S7u[row, r + 24] = (2.0 ** i) * INV2PI
        S7Tu[row, r] = S7Tu[row, r + 24] = (2.0 ** i) * INV2PI
        if j < 3:
            Sd7[3 + j, r] = 2.0 ** i
            Sd7[3 + j, r + 24] = -(2.0 ** i)
    S7u[7, 24:48] = 0.25                      # cos rows shift
    S7Tu[7, 0:24] = 0.25                      # T = [cos; sin]
    wp.add("S7u", S7u)
    wp.add("S7Tu", S7Tu)
    wp.add("Sd7", Sd7)

    # ---------------- sdf  (enc order [trig36; lin3], ct_enc at base 64)
    s2x, c2x = _trig_perm(3, 6, 0)
    strig_rows = s2x + c2x                    # 36
    wp.add("s_W0_trig", Ws[0][strig_rows, :])
    W0lin4 = np.zeros((4, 256), np.float32)
    W0lin4[0:3] = Ws[0][0:3]
    W0lin4[3] = bs[0]
    wp.add("s_W0_lin4", W0lin4)
    for l in (1, 2, 3, 4, 6, 7):
        wp.add(f"s_W{l}_k0", Ws[l][0:128])
        wp.add(f"s_W{l}_k1", Ws[l][128:256])
    S5 = Ws[5] / SQRT2
    wp.add("s_W5_k0", S5[0:128])
    wp.add("s_W5_k1", S5[128:256])
    wp.add("s_W5_trig", S5[256:295][strig_rows, :])
    W5lin4 = np.zeros((4, 256), np.float32)
    W5lin4[0:3] = S5[256:295][0:3]
    W5lin4[3] = bs[5]
    wp.add("s_W5_lin4", W5lin4)
    # L8 column splits: [0], [1:129], [129:257]
    wp.add("s_W8_sdf_k0", Ws[8][0:128, 0:1])
    wp.add("s_W8_sdf_k1", Ws[8][128:256, 0:1])
    wp.add("s_W8_g1_k0", Ws[8][0:128, 1:129])
    wp.add("s_W8_g1_k1", Ws[8][128:256, 1:129])
    wp.add("s_W8_g2_k0", Ws[8][0:128, 129:257])
    wp.add("s_W8_g2_k1", Ws[8][128:256, 129:257])
    # backward (negated true weights)
    for l in (1, 2, 3, 4, 6, 7):
        V = -(Ws[l].T)
        wp.add(f"s_V{l}_k0", V[0:128])
        wp.add(f"s_V{l}_k1", V[128:256])
    V5 = -(Ws[5] / SQRT2).T                   # [256, 295]
    wp.add("s_V5h_k0", V5[0:128, 0:256])
    wp.add("s_V5h_k1", V5[128:256, 0:256])
    enc_cols = strig_rows + [0, 1, 2]
    V5e = V5[:, 256:][:, enc_cols]
    wp.add("s_V5e_k0", V5e[0:128])
    wp.add("s_V5e_k1", V5e[128:256])
    V0 = -(Ws[0].T)[:, enc_cols]
    wp.add("s_V0_k0", V0[0:128])
    wp.add("s_V0_k1", V0[128:256])
    # G trig part at base 64 (AB rows 64..99), lin part at base 0
    G = np.zeros((36, 3), np.float32)
    for r in range(18):
        j, i = r // 6, r % 6
        G[r, j] = 2.0 ** i
        G[18 + r, j] = -(2.0 ** i)
    wp.add("G64", G, base=64)
    wp.add("G_lin", np.eye(3, dtype=np.float32))
    # arg matrices (rhs = XC4 [x_c;ones] or DC4)
    S2u = np.zeros((4, 36), np.float32)
    S2Tu = np.zeros((4, 36), np.float32)
    for r in range(18):
        j, i = r // 6, r % 6
        S2u[j, r] = S2u[j, r + 18] = (2.0 ** i) * INV2PI
        S2Tu[j, r] = S2Tu[j, r + 18] = (2.0 ** i) * INV2PI
    S2u[3, 18:36] = 0.25
    S2Tu[3, 0:18] = 0.25
    wp.add("S2u", S2u)
    wp.add("S2Tu64", S2Tu)                    # out at base 64 (T2)
    wp.add("ones3", np.ones((3, 1), np.float32))
    wp.add("bcast13", np.ones((1, 3), np.float32))

    # ---------------- color (input chunks: trig3@64, trig4, XC4, DC3, GRAD, geo)
    s3x, c3x = _trig_perm(3, 10, 0)
    trig3_rows = s3x + c3x                    # 60
    s4x, c4x = _trig_perm(3, 4, 66)
    trig4_rows = s4x + c4x                    # 24
    W0 = Wc[0]
    wp.add("c_W0_trig3_64", W0[trig3_rows, :], base=64)
    wp.add("c_W0_trig4", W0[trig4_rows, :])
    cW0lin4 = np.zeros((4, 256), np.float32)
    cW0lin4[0:3] = W0[0:3]                    # x_c lin rows
    cW0lin4[3] = bc[0]                        # bias fold
    wp.add("c_W0_xc4", cW0lin4)
    wp.add("c_W0_dc", W0[66:69, :])
    wp.add("c_W0_grad", W0[63:66, :])
    wp.add("c_W0_geo_k0", W0[93:221, :])
    wp.add("c_W0_geo_k1", W0[221:349, :])
    for l in (1, 2, 3):
        wp.add(f"c_W{l}_k0", Wc[l][0:128])
        wp.add(f"c_W{l}_k1", Wc[l][128:256])
    wp.add("c_W4_k0", Wc[4][0:128])           # [128, 3]
    wp.add("c_W4_k1", Wc[4][128:256])
    S3u = np.zeros((4, 60), np.float32)
    for r in range(30):
        j, i = r // 10, r % 10
        S3u[j, r] = S3u[j, r + 30] = (2.0 ** i) * INV2PI
    S3u[3, 30:60] = 0.25
    wp.add("S3u64", S3u)                      # args3 out at base 64
    S4u = np.zeros((4, 24), np.float32)
    for r in range(12):
        j, i = r // 4, r % 4
        S4u[j, r] = S4u[j, r + 12] = (2.0 ** i) * INV2PI
    S4u[3, 12:24] = 0.25
    wp.add("S4u", S4u)

    # ---------------- consts pack (per-partition bias vectors, [*, 1])
    for l in range(1, 8):
        cp.add(f"d_b{l}_c0", bd[l][0:128, None])
        cp.add(f"d_b{l}_c1", bd[l][128:256, None])
    cp.add("d_b8", bd[8][:, None])            # [3,1]
    for l in (1, 2, 3, 4, 6, 7):
        cp.add(f"s_b{l}_c0", bs[l][0:128, None])
        cp.add(f"s_b{l}_c1", bs[l][128:256, None])
        cp.add(f"s_b{l}100_c0", 100.0 * bs[l][0:128, None])
        cp.add(f"s_b{l}100_c1", 100.0 * bs[l][128:256, None])
    cp.add("s_b8_sdf", bs[8][0:1, None])
    cp.add("s_b8_g1", bs[8][1:129, None])
    cp.add("s_b8_g2", bs[8][129:257, None])
    cp.add("s_w8col0_c0", Ws[8][0:128, 0:1])
    cp.add("s_w8col0_c1", Ws[8][128:256, 0:1])
    for l in (1, 2, 3):
        cp.add(f"c_b{l}_c0", bc[l][0:128, None])
        cp.add(f"c_b{l}_c1", bc[l][128:256, None])
    cp.add("c_b4_half", 0.5 * bc[4][:, None])  # tanh bias
    return wp, cp


# ===================== device program ======================================

_CACHE = {}


def build_program(wp_entries, cp_entries, wcols, ccols):
    """Build the Bass program. wp_entries/cp_entries: name -> (off, base, K, M)."""
    nc = bacc.Bacc("TRN2", target_bir_lowering=False, debug=False,
                   num_devices=NCORES)

    inp_d = nc.dram_tensor("inputs", [NPC, 7], F32, kind="ExternalInput")
    wpack_d = nc.dram_tensor("wpack", [128, wcols], F32, kind="ExternalInput")
    cpack_d = nc.dram_tensor("cpack", [128, ccols], F32, kind="ExternalInput")
    invs_d = nc.dram_tensor("invs", [1, P], F32, kind="ExternalInput")
    out_d = nc.dram_tensor("out", [NPC, 8], F32, kind="ExternalOutput")

    act_prev = [None]

    def act(*args, **kwargs):
        ins = nc.scalar.activation(*args, **kwargs).ins
        if act_prev[0] is not None:
            tile.add_dep_helper(ins, act_prev[0], reason="act table order")
        act_prev[0] = ins
        return ins

    with tile.TileContext(nc) as tc, ExitStack() as ctx:
        konst = ctx.enter_context(tc.tile_pool(name="konst", bufs=1))
        wt = konst.tile([128, wcols], F32, name="wt")
        ct = konst.tile([128, ccols], F32, name="ct")
        invs = konst.tile([1, P], F32, name="invs")
        nc.sync.dma_start(out=wt, in_=wpack_d.ap())
        nc.sync.dma_start(out=ct, in_=cpack_d.ap())
        nc.sync.dma_start(out=invs, in_=invs_d.ap())

        def W(name, msl=None):
            off, base, K, M = wp_entries[name]
            if msl is None:
                return wt[base:base + K, off:off + M]
            return wt[base:base + K, off + msl[0]:off + msl[1]]

        def C(name):
            off, base, K, M = cp_entries[name]
            assert M == 1
            return ct[base:base + K, off:off + 1]

        saves = ctx.enter_context(tc.tile_pool(name="saves", bufs=1))
        hsave = saves.tile([128, 16 * P], F32, name="hsave")  # sdf h' 8 layers x 2 chunks

        def hs(l, c):
            return hsave[:, (l * 2 + c) * P:(l * 2 + c + 1) * P]

        work = ctx.enter_context(tc.tile_pool(name="work", bufs=2))
        pwork = ctx.enter_context(tc.tile_pool(name="pwork", bufs=1, space="PSUM"))

        for t in range(NT):
            sl = slice(t * P, (t + 1) * P)

            # ---------------- inputs
            xdt = work.tile([8, P], F32, name="xdt", tag="xdt", bufs=2)
            nc.vector.memset(xdt, 1.0)
            nc.sync.dma_start(out=xdt[0:7, :], in_=inp_d.ap()[sl, :].rearrange("n f -> f n"))
            ddup = work.tile([3, P], F32, name="ddup", tag="ddup", bufs=1)
            nc.sync.dma_start(out=ddup, in_=inp_d.ap()[sl, 3:6].rearrange("n f -> f n"))

            # ---------------- deform encoding
            A1 = pwork.tile([112, P], F32, name="A1", tag="argps", bufs=1)
            nc.tensor.matmul(A1[0:48, :], W("S7u"), xdt, start=True, stop=True)
            nc.tensor.matmul(A1[64:112, :], W("S7Tu"), xdt, start=True, stop=True)
            V1 = work.tile([112, P], F32, name="V1", tag="vred", bufs=2)
            nc.vector.tensor_scalar(V1, A1, MAGIC, MAGIC, ALU.add, ALU.subtract)
            W1 = work.tile([112, P], F32, name="W1", tag="wred", bufs=2)
            nc.vector.scalar_tensor_tensor(W1, V1, -1.0, A1, ALU.mult, ALU.add)
            TRIG1 = work.tile([112, P], F32, name="TRIG1", tag="trig1", bufs=1)
            act(TRIG1, W1, AF.Sin, bias=0.0, scale=TWOPI)
            A2 = pwork.tile([112, P], F32, name="A2", tag="argps", bufs=1)
            nc.tensor.matmul(A2[64:112, :], W("Sd7"), xdt, start=True, stop=True)
            TDOT = work.tile([112, P], F32, name="TDOT", tag="tdot", bufs=1)
            nc.vector.tensor_tensor(TDOT[64:112, :], TRIG1[64:112, :], A2[64:112, :], ALU.mult)

            # ---------------- deform MLP fwd + JVP
            h = [None, None]
            hd = [None, None]
            for l in range(8):
                ph = [pwork.tile([128, P], F32, name=f"ph{l}_{m}", tag=f"ph{m}", bufs=2)
                      for m in range(2)]
                pd = [pwork.tile([128, P], F32, name=f"pd{l}_{m}", tag=f"pd{m}", bufs=1)
                      for m in range(2)]
                for m in range(2):
                    msl = (m * 128, (m + 1) * 128)
                    if l == 0:
                        nc.tensor.matmul(ph[m], W("d_W0_trig", msl), TRIG1[0:48, :], start=True, stop=False)
                        nc.tensor.matmul(ph[m], W("d_W0_lin8", msl), xdt, start=False, stop=True)
                        nc.tensor.matmul(pd[m], W("d_W0_trig64", msl), TDOT[64:112, :], start=True, stop=False)
                        nc.tensor.matmul(pd[m], W("d_W0_lint8", msl), xdt, start=False, stop=True)
                    elif l == 5:
                        nc.tensor.matmul(ph[m], W("d_W5_k0", msl), h[0], start=True, stop=False)
                        nc.tensor.matmul(ph[m], W("d_W5_k1", msl), h[1], start=False, stop=False)
                        nc.tensor.matmul(ph[m], W("d_W5_trig", msl), TRIG1[0:48, :], start=False, stop=False)
                        nc.tensor.matmul(ph[m], W("d_W5_lin8", msl), xdt, start=False, stop=True)
                        nc.tensor.matmul(pd[m], W("d_W5_k0", msl), hd[0], start=True, stop=False)
                        nc.tensor.matmul(pd[m], W("d_W5_k1", msl), hd[1], start=False, stop=False)
                        nc.tensor.matmul(pd[m], W("d_W5_trig64", msl), TDOT[64:112, :], start=False, stop=False)
                        nc.tensor.matmul(pd[m], W("d_W5_lint8", msl), xdt, start=False, stop=True)
                    else:
                        nc.tensor.matmul(ph[m], W(f"d_W{l}_k0", msl), h[0], start=True, stop=False)
                        nc.tensor.matmul(ph[m], W(f"d_W{l}_k1", msl), h[1], start=False, stop=True)
                        nc.tensor.matmul(pd[m], W(f"d_W{l}_k0", msl), hd[0], start=True, stop=False)
                        nc.tensor.matmul(pd[m], W(f"d_W{l}_k1", msl), hd[1], start=False, stop=True)
                nh = [work.tile([128, P], F32, name=f"h{l}_{m}", tag=f"h{m}", bufs=2) for m in range(2)]
                nhd = [work.tile([128, P], F32, name=f"hd{l}_{m}", tag=f"hdt{m}", bufs=2) for m in range(2)]
                for m in range(2):
                    if l in (0, 5):
                        act(nh[m], ph[m], AF.Relu)          # bias folded via ones row
                    else:
                        act(nh[m], ph[m], AF.Relu, bias=C(f"d_b{l}_c{m}"), scale=1.0)
                    nc.vector.scalar_tensor_tensor(nhd[m], nh[m], 0.0, pd[m], ALU.is_gt, ALU.mult)
                h, hd = nh, nhd

            ph8 = pwork.tile([3, P], F32, name="ph8", tag="argps", bufs=1)
            nc.tensor.matmul(ph8, W("d_W8_k0"), h[0], start=True, stop=False)
            nc.tensor.matmul(ph8, W("d_W8_k1"), h[1], start=False, stop=True)
            pd8 = pwork.tile([3, P], F32, name="pd8", tag="argps", bufs=1)
            nc.tensor.matmul(pd8, W("d_W8_k0"), hd[0], start=True, stop=False)
            nc.tensor.matmul(pd8, W("d_W8_k1"), hd[1], start=False, stop=True)

            XC4 = work.tile([4, P], F32, name="XC4", tag="xc4", bufs=1)
            nc.vector.memset(XC4, 1.0)
            nc.vector.scalar_tensor_tensor(XC4[0:3, :], ph8, C("d_b8"), xdt[0:3, :], ALU.add, ALU.add)
            DCU = work.tile([3, P], F32, name="DCU", tag="dcu", bufs=1)
            nc.vector.tensor_tensor(DCU, pd8, ddup, ALU.add)

            # ---------------- x_c encodings (sin block 1, set 18)
            X1 = pwork.tile([124, P], F32, name="X1", tag="argps", bufs=1)
            nc.tensor.matmul(X1[0:36, :], W("S2u"), XC4, start=True, stop=True)
            nc.tensor.matmul(X1[64:124, :], W("S3u64"), XC4, start=True, stop=True)
            V2 = work.tile([124, P], F32, name="V2", tag="vred", bufs=2)
            nc.vector.tensor_scalar(V2, X1, MAGIC, MAGIC, ALU.add, ALU.subtract)
            W2 = work.tile([124, P], F32, name="W2", tag="wred", bufs=2)
            nc.vector.scalar_tensor_tensor(W2, V2, -1.0, X1, ALU.mult, ALU.add)
            TRIGX = work.tile([124, P], F32, name="TRIGX", tag="trigx", bufs=1)
            act(TRIGX, W2, AF.Sin, bias=0.0, scale=TWOPI)
            # T2 (cos/sin swapped) at base 64 for the vjp AB product
            X2 = pwork.tile([100, P], F32, name="X2", tag="argps", bufs=1)
            nc.tensor.matmul(X2[64:100, :], W("S2Tu64"), XC4, start=True, stop=True)
            V3 = work.tile([100, P], F32, name="V3", tag="vred", bufs=2)
            nc.vector.tensor_scalar(V3[64:100, :], X2[64:100, :], MAGIC, MAGIC, ALU.add, ALU.subtract)
            W3 = work.tile([100, P], F32, name="W3", tag="wred", bufs=2)
            nc.vector.scalar_tensor_tensor(W3[64:100, :], V3[64:100, :], -1.0, X2[64:100, :], ALU.mult, ALU.add)
            T2 = work.tile([100, P], F32, name="T2", tag="t2", bufs=1)
            act(T2[64:100, :], W3[64:100, :], AF.Sin, bias=0.0, scale=TWOPI)

            # ---------------- sdf fwd (set 6: exp/ln)
            sh = [None, None]
            for l in range(8):
                ph = [pwork.tile([128, P], F32, name=f"sph{l}_{m}", tag=f"ph{m}", bufs=2)
                      for m in range(2)]
                for m in range(2):
                    msl = (m * 128, (m + 1) * 128)
                    if l == 0:
                        nc.tensor.matmul(ph[m], W("s_W0_trig", msl), TRIGX[0:36, :], start=True, stop=False)
                        nc.tensor.matmul(ph[m], W("s_W0_lin4", msl), XC4, start=False, stop=True)
                    elif l == 5:
                        nc.tensor.matmul(ph[m], W("s_W5_k0", msl), sh[0], start=True, stop=False)
                        nc.tensor.matmul(ph[m], W("s_W5_k1", msl), sh[1], start=False, stop=False)
                        nc.tensor.matmul(ph[m], W("s_W5_trig", msl), TRIGX[0:36, :], start=False, stop=False)
                        nc.tensor.matmul(ph[m], W("s_W5_lin4", msl), XC4, start=False, stop=True)
                    else:
                        nc.tensor.matmul(ph[m], W(f"s_W{l}_k0", msl), sh[0], start=True, stop=False)
                        nc.tensor.matmul(ph[m], W(f"s_W{l}_k1", msl), sh[1], start=False, stop=True)
                nsh = [hs(l, m) for m in range(2)]
                for m in range(2):
                    az = work.tile([128, P], F32, name=f"az{l}_{m}", tag=f"az{m}", bufs=1)
                    EE = work.tile([128, P], F32, name=f"E{l}_{m}", tag=f"E{m}", bufs=2)
                    LL = work.tile([128, P], F32, name=f"L{l}_{m}", tag=f"LL{m}", bufs=1)
                    rz = work.tile([128, P], F32, name=f"rz{l}_{m}", tag=f"rz{m}", bufs=1)
                    if l in (0, 5):
                        act(az, ph[m], AF.Abs, bias=0.0, scale=100.0)
                        nc.vector.tensor_scalar(rz, ph[m], 0.0, 0.0, ALU.add, ALU.max)
                    else:
                        act(az, ph[m], AF.Abs, bias=C(f"s_b{l}100_c{m}"), scale=100.0)
                        nc.vector.tensor_scalar(rz, ph[m], C(f"s_b{l}_c{m}"), 0.0, ALU.add, ALU.max)
                    act(EE, az, AF.Exp, bias=0.0, scale=-1.0)
                    act(LL, EE, AF.Ln, bias=1.0, scale=1.0)
                    nc.vector.scalar_tensor_tensor(nsh[m], LL, 0.01, rz, ALU.mult, ALU.add)
                sh = nsh

            psdf = pwork.tile([1, P], F32, name="psdf", tag="argps", bufs=1)
            nc.tensor.matmul(psdf, W("s_W8_sdf_k0"), sh[0], start=True, stop=False)
            nc.tensor.matmul(psdf, W("s_W8_sdf_k1"), sh[1], start=False, stop=True)
            SDFV = work.tile([1, P], F32, name="SDFV", tag="sdfv", bufs=1)
            nc.vector.tensor_scalar(SDFV, psdf, C("s_b8_sdf"), 0.0, ALU.add, ALU.add)
            GEO = [work.tile([128, P], F32, name=f"GEO{m}", tag=f"geo{m}", bufs=1) for m in range(2)]
            for m, nm in ((0, "g1"), (1, "g2")):
                pg = pwork.tile([128, P], F32, name=f"pg{m}", tag=f"ph{m}", bufs=2)
                nc.tensor.matmul(pg, W(f"s_W8_{nm}_k0"), sh[0], start=True, stop=False)
                nc.tensor.matmul(pg, W(f"s_W8_{nm}_k1"), sh[1], start=False, stop=True)
                nc.vector.tensor_scalar(GEO[m], pg, C(f"s_b8_{nm}"), 0.0, ALU.add, ALU.add)

            # ---------------- sdf vjp (set 6: exp)
            ctv = [None, None]
            ctenc = pwork.tile([100, P], F32, name="ctenc", tag="ctenc", bufs=1)
            for l in range(7, -1, -1):
                ctX = [work.tile([128, P], F32, name=f"ctX{l}_{m}", tag=f"ctx{m}", bufs=2)
                       for m in range(2)]
                for m in range(2):
                    EP = work.tile([128, P], F32, name=f"EP{l}_{m}", tag=f"E{m}", bufs=2)
                    act(EP, hs(l, m), AF.Exp, bias=0.0, scale=-100.0)
                    if l == 7:
                        nc.vector.tensor_scalar(ctX[m], EP, 1.0, C(f"s_w8col0_c{m}"),
                                                ALU.subtract, ALU.mult)
                    else:
                        nc.vector.scalar_tensor_tensor(ctX[m], EP, 1.0, ctv[m],
                                                       ALU.subtract, ALU.mult)
                if l == 0:
                    nc.tensor.matmul(ctenc[64:100, :], W("s_V0_k0", (0, 36)), ctX[0], start=False, stop=False)
                    nc.tensor.matmul(ctenc[64:100, :], W("s_V0_k1", (0, 36)), ctX[1], start=False, stop=True)
                    nc.tensor.matmul(ctenc[0:3, :], W("s_V0_k0", (36, 39)), ctX[0], start=False, stop=False)
                    nc.tensor.matmul(ctenc[0:3, :], W("s_V0_k1", (36, 39)), ctX[1], start=False, stop=True)
                    break
                nctp = [pwork.tile([128, P], F32, name=f"ctp{l}_{m}", tag=f"pd{m}", bufs=1)
                        for m in range(2)]
                key = "s_V5h" if l == 5 else f"s_V{l}"
                for m in range(2):
                    msl = (m * 128, (m + 1) * 128)
                    nc.tensor.matmul(nctp[m], W(f"{key}_k0", msl), ctX[0], start=True, stop=False)
                    nc.tensor.matmul(nctp[m], W(f"{key}_k1", msl), ctX[1], start=False, stop=True)
                if l == 5:
                    nc.tensor.matmul(ctenc[64:100, :], W("s_V5e_k0", (0, 36)), ctX[0], start=True, stop=False)
                    nc.tensor.matmul(ctenc[64:100, :], W("s_V5e_k1", (0, 36)), ctX[1], start=False, stop=False)
                    nc.tensor.matmul(ctenc[0:3, :], W("s_V5e_k0", (36, 39)), ctX[0], start=True, stop=False)
                    nc.tensor.matmul(ctenc[0:3, :], W("s_V5e_k1", (36, 39)), ctX[1], start=False, stop=False)
                ctv = nctp

            STACK = work.tile([100, P], F32, name="STACK", tag="stack", bufs=1)
            nc.vector.tensor_tensor(STACK[64:100, :], ctenc[64:100, :], T2[64:100, :], ALU.mult)
            CTLIN = work.tile([3, P], F32, name="CTLIN", tag="ctlin", bufs=1)
            nc.vector.tensor_copy(CTLIN, ctenc[0:3, :])
            pgr = pwork.tile([3, P], F32, name="pgr", tag="argps", bufs=1)
            nc.tensor.matmul(pgr, W("G64"), STACK[64:100, :], start=True, stop=False)
            nc.tensor.matmul(pgr, W("G_lin"), CTLIN, start=False, stop=True)
            GRAD = work.tile([3, P], F32, name="GRAD", tag="grad", bufs=1)
            nc.vector.tensor_copy(GRAD, pgr)

            # ---------------- normalize d_c (set 6: ln/exp)
            SQ = work.tile([3, P], F32, name="SQ", tag="sq", bufs=1)
            act(SQ, DCU, AF.Square)
            pn2 = pwork.tile([1, P], F32, name="pn2", tag="argps", bufs=1)
            nc.tensor.matmul(pn2, W("ones3"), SQ, start=True, stop=True)
            LN2 = work.tile([1, P], F32, name="LN2", tag="ln2", bufs=1)
            act(LN2, pn2, AF.Ln)
            RIN = work.tile([1, P], F32, name="RIN", tag="rin", bufs=1)
            act(RIN, LN2, AF.Exp, bias=0.0, scale=-0.5)
            pbc = pwork.tile([3, P], F32, name="pbc", tag="argps", bufs=1)
            nc.tensor.matmul(pbc, W("bcast13"), RIN, start=True, stop=True)
            DC4 = work.tile([4, P], F32, name="DC4", tag="dc4", bufs=1)
            nc.vector.memset(DC4, 1.0)
            nc.vector.tensor_tensor(DC4[0:3, :], pbc, DCU, ALU.mult)

            # ---------------- d_c encoding (sin block 2, set 18)
            A4 = pwork.tile([24, P], F32, name="A4", tag="argps", bufs=1)
            nc.tensor.matmul(A4, W("S4u"), DC4, start=True, stop=True)
            V4 = work.tile([24, P], F32, name="V4", tag="vred", bufs=2)
            nc.vector.tensor_scalar(V4, A4, MAGIC, MAGIC, ALU.add, ALU.subtract)
            W4 = work.tile([24, P], F32, name="W4", tag="wred", bufs=2)
            nc.vector.scalar_tensor_tensor(W4, V4, -1.0, A4, ALU.mult, ALU.add)
            TRIG4 = work.tile([24, P], F32, name="TRIG4", tag="trig4", bufs=1)
            act(TRIG4, W4, AF.Sin, bias=0.0, scale=TWOPI)

            # ---------------- color MLP
            chv = [None, None]
            for l in range(4):
                ph = [pwork.tile([128, P], F32, name=f"cph{l}_{m}", tag=f"ph{m}", bufs=2)
                      for m in range(2)]
                for m in range(2):
                    msl = (m * 128, (m + 1) * 128)
                    if l == 0:
                        nc.tensor.matmul(ph[m], W("c_W0_trig3_64", msl), TRIGX[64:124, :], start=True, stop=False)
                        nc.tensor.matmul(ph[m], W("c_W0_trig4", msl), TRIG4, start=False, stop=False)
                        nc.tensor.matmul(ph[m], W("c_W0_xc4", msl), XC4, start=False, stop=False)
                        nc.tensor.matmul(ph[m], W("c_W0_dc", msl), DC4[0:3, :], start=False, stop=False)
                        nc.tensor.matmul(ph[m], W("c_W0_grad", msl), GRAD, start=False, stop=False)
                        nc.tensor.matmul(ph[m], W("c_W0_geo_k0", msl), GEO[0], start=False, stop=False)
                        nc.tensor.matmul(ph[m], W("c_W0_geo_k1", msl), GEO[1], start=False, stop=True)
                    else:
                        nc.tensor.matmul(ph[m], W(f"c_W{l}_k0", msl), chv[0], start=True, stop=False)
                        nc.tensor.matmul(ph[m], W(f"c_W{l}_k1", msl), chv[1], start=False, stop=True)
                nh = [work.tile([128, P], F32, name=f"ch{l}_{m}", tag=f"h{m}", bufs=2) for m in range(2)]
                for m in range(2):
                    if l == 0:
                        act(nh[m], ph[m], AF.Relu)
                    else:
                        act(nh[m], ph[m], AF.Relu, bias=C(f"c_b{l}_c{m}"), scale=1.0)
                chv = nh
            pcol = pwork.tile([3, P], F32, name="pcol", tag="argps", bufs=1)
            nc.tensor.matmul(pcol, W("c_W4_k0"), chv[0], start=True, stop=False)
            nc.tensor.matmul(pcol, W("c_W4_k1"), chv[1], start=False, stop=True)
            TH = work.tile([3, P], F32, name="TH", tag="th", bufs=1)
            act(TH, pcol, AF.Tanh, bias=C("c_b4_half"), scale=0.5)
            COL = work.tile([3, P], F32, name="COL", tag="col", bufs=1)
            nc.vector.tensor_scalar(COL, TH, 0.5, 0.5, ALU.mult, ALU.add)

            # ---------------- outputs
            nc.sync.dma_start(out=out_d.ap()[sl, 0:3].rearrange("n f -> f n"), in_=COL)
            nc.sync.dma_start(out=out_d.ap()[sl, 3:4].rearrange("n f -> f n"), in_=SDFV)
            nc.sync.dma_start(out=out_d.ap()[sl, 4:5].rearrange("n f -> f n"), in_=invs)
            nc.sync.dma_start(out=out_d.ap()[sl, 5:8].rearrange("n f -> f n"), in_=GRAD)

    nc.finalize()
    return nc


# ===================== entry point =========================================

def kernel(inputs, deform_params, sdf_params, color_params, variance):
    inputs = np.ascontiguousarray(np.asarray(inputs, np.float32))
    deform_params = [(np.asarray(w, np.float32), np.asarray(b, np.float32))
                     for (w, b) in deform_params]
    sdf_params = [(np.asarray(w, np.float32), np.asarray(b, np.float32))
                  for (w, b) in sdf_params]
    color_params = [(np.asarray(w, np.float32), np.asarray(b, np.float32))
                    for (w, b) in color_params]
    variance = np.float32(np.asarray(variance))

    wp, cp = build_packs(deform_params, sdf_params, color_params)
    wpack = wp.materialize()
    cpack = cp.materialize()
    invs_val = np.exp(np.float32(10.0) * variance).astype(np.float32)
    invs_row = np.full((1, P), invs_val, np.float32)

    key = ("prog", wpack.shape[1], cpack.shape[1])
    if key not in _CACHE:
        _CACHE[key] = build_program(wp.entries, cp.entries,
                                    wpack.shape[1], cpack.shape[1])
    nc = _CACHE[key]

    in_maps = []
    for c in range(NCORES):
        in_maps.append({
            "inputs": np.ascontiguousarray(inputs[c * NPC:(c + 1) * NPC]),
            "wpack": wpack,
            "cpack": cpack,
            "invs": invs_row,
        })
    res = run_bass_kernel_spmd(nc, in_maps, core_ids=list(range(NCORES)))
    out = np.concatenate([res.results[c]["out"] for c in range(NCORES)], 0)
    return out.astype(np.float32)


# revision 16
# speedup vs baseline: 21.2070x; 21.2070x over previous
"""Trainium2 Bass kernel for nn_EndoSurfRenderer (EndoSurf-style renderer).

Contract: kernel(**inputs) takes the FULL inputs from setup_inputs() and
returns the FULL [65536, 8] output. Internally shards points across 8
NeuronCores (data parallel), replicates weights, runs one fused Bass/Tile
kernel per core, and gathers.

Pipeline per point-tile (feature-major on-chip layout [features, points]):
  deform MLP fwd + JVP tangent -> x_c, d_c ; sdf MLP fwd (softplus beta=100
  built from exp+ln) ; sdf VJP (sigmoid = 1-exp(-100*h')) -> gradients ;
  d_c normalize via exp(-0.5*ln(|d|^2)) ; color MLP ; assemble [N, 8].

All sin/cos computed as sin(2*pi*w) after exact range reduction
w = u - round(u), u = arg/(2*pi), round via the 1.5*2^23 magic constant.
All matmuls fp32 (fp32r is broken on this silicon; bf16 insufficient for
the beta=100 softplus sensitivity).
"""

import sys
from contextlib import ExitStack

if "/opt/trn_rl_repo" not in sys.path:
    sys.path.insert(0, "/opt/trn_rl_repo")

import numpy as np

import concourse.bacc as bacc
import concourse.tile as tile
from concourse import mybir
from concourse.bass_utils import run_bass_kernel_spmd
import concourse.hw_specs as _hw_specs

_orig_get_tables = _hw_specs.get_activation_tables


def _patched_get_tables(arch):
    tabs = dict(_orig_get_tables(arch))
    out = {}
    target6 = "natural_log_exp_and_others"
    target18 = "silu_and_others"
    move6 = set()
    move18 = set()
    for name, funcs in tabs.items():
        if name == target6:
            move6 = set(funcs)
        if name == target18:
            move18 = set(funcs)
    for name, funcs in tabs.items():
        if name == target6 or name == target18:
            out[name] = funcs
        else:
            keep = {f for f in funcs if f not in move6 and f not in move18}
            out[name] = keep
    return out


_hw_specs.get_activation_tables = _patched_get_tables
bacc.get_activation_tables = _patched_get_tables

F32 = mybir.dt.float32
BF16 = mybir.dt.bfloat16
AF = mybir.ActivationFunctionType
ALU = mybir.AluOpType

NPTS = 65536
NCORES = 8
NPC = NPTS // NCORES        # 8192 points per core
P = 512                     # points per tile
NT = NPC // P               # tiles per core
MAGIC = float(1.5 * 2 ** 23)
TWOPI = float(2.0 * np.pi)
INV2PI = float(1.0 / (2.0 * np.pi))
SQRT2 = np.float32(np.sqrt(2.0))

# tunables (swept via timeline sim)
PH_BUFS = 2
PD_BUFS = 1
PD_TAG = "ph"            # "ph" shares psum tags between h and hd
ARG_BUFS = 2
H_BUFS = 2
E_BUFS = 2
CTX_BUFS = 2
ACT_CHAIN = False
BF16_VJP = True
BF16_COLOR = True
SMALL_BUFS = 1
LATE_BUFS = 2
TRANS_BUFS = 1


# ===================== host-side packing (layout registry) ==================

class Pack:
    """Packs many [K, M] weight blocks into one [128, cols] f32 array.
    Each entry is placed at partition rows [base, base+K) and a column
    range; on-device it is read as an SBUF AP slice of one big tile."""

    def __init__(self):
        self.cols = 0
        self.entries = {}   # name -> (off, base, K, M)
        self.arrays = {}

    def add(self, name, arr, base=0, share=None):
        arr = np.ascontiguousarray(arr, np.float32)
        assert arr.ndim == 2
        K, M = arr.shape
        assert base + K <= 128, (name, base, K)
        if share is not None:
            soff, sbase, sK, sM = self.entries[share]
            assert M <= sM and (base >= sbase + sK or base + K <= sbase), (name, share)
            self.entries[name] = (soff, base, K, M)
        else:
            self.entries[name] = (self.cols, base, K, M)
        self.cols += 0 if share is not None else M
        self.arrays[name] = arr

    def materialize(self, dtype=np.float32):
        w = np.zeros((128, max(self.cols, 1)), dtype)
        for name, (off, base, K, M) in self.entries.items():
            w[base:base + K, off:off + M] = self.arrays[name].astype(dtype)
        return w


def _trig_perm(D, L, base):
    sin_idx = [base + D + j * 2 * L + i for j in range(D) for i in range(L)]
    cos_idx = [base + D + j * 2 * L + L + i for j in range(D) for i in range(L)]
    return sin_idx, cos_idx


def build_packs(deform_params, sdf_params, color_params):
    """Returns (wpack, cpack, bpack) Pack objects with every weight block."""
    wp = Pack()
    cp = Pack()
    bp = Pack()

    Wd = [np.asarray(w, np.float32) for (w, b) in deform_params]
    bd = [np.asarray(b, np.float32) for (w, b) in deform_params]
    Ws = [np.asarray(w, np.float32) for (w, b) in sdf_params]
    bs = [np.asarray(b, np.float32) for (w, b) in sdf_params]
    Wc = [np.asarray(w, np.float32) for (w, b) in color_params]
    bc = [np.asarray(b, np.float32) for (w, b) in color_params]

    # ---------------- deform enc helpers
    sx, cx = _trig_perm(3, 6, 0)
    st_, ct_ = _trig_perm(1, 6, 39)
    dtrig_rows = sx + st_ + cx + ct_          # 48: sin(x0..2,t) then cos(...)

    def split_dW0(W):
        trig = W[dtrig_rows, :]
        lin8 = np.zeros((8, W.shape[1]), np.float32)
        lin8[0:3] = W[0:3]
        lin8[6] = W[39]
        lint8 = np.zeros((8, W.shape[1]), np.float32)
        lint8[3:6] = W[0:3]
        return trig, lin8, lint8

    t0, l0, lt0 = split_dW0(Wd[0])
    l0[7] = bd[0]                             # bias via ones-row
    wp.add("d_W0_trig", t0)
    wp.add("d_W0_trig64", t0, base=64, share="d_W0_trig")
    wp.add("d_W0_lin8", l0)
    wp.add("d_W0_lint8", lt0)
    for l in (1, 2, 3, 4, 6, 7):
        wp.add(f"d_W{l}_k0", Wd[l][0:128])
        wp.add(f"d_W{l}_k1", Wd[l][128:256])
    W5 = Wd[5] / SQRT2
    wp.add("d_W5_k0", W5[0:128])
    wp.add("d_W5_k1", W5[128:256])
    t5, l5, lt5 = split_dW0(W5[256:308])
    l5[7] = bd[5]
    wp.add("d_W5_trig", t5)
    wp.add("d_W5_trig64", t5, base=64, share="d_W5_trig")
    wp.add("d_W5_lin8", l5)
    wp.add("d_W5_lint8", lt5)
    wp.add("d_W8_k0", Wd[8][0:128])           # [128, 3]
    wp.add("d_W8_k1", Wd[8][128:256])

    # S matrices (args in u = arg/2pi space, ones-row carries +0.25 cos shift)
    S7u = np.zeros((8, 48), np.float32)
    S7Tu = np.zeros((8, 48), np.float32)
    Sd7 = np.zeros((8, 48), np.float32)
    for r in range(24):
        j, i = r // 6, r % 6
        row = 6 if j == 3 else j
        S7u[row, r] = S7u[row, r + 24] = (2.0 ** i) * INV2PI
        S7Tu[row, r] = S7Tu[row, r + 24] = (2.0 ** i) * INV2PI
        if j < 3:
            Sd7[3 + j, r] = 2.0 ** i
            Sd7[3 + j, r + 24] = -(2.0 ** i)
    S7u[7, 24:48] = 0.25                      # cos rows shift
    S7Tu[7, 0:24] = 0.25                      # T = [cos; sin]
    wp.add("S7u", S7u)
    wp.add("S7Tu", S7Tu)
    wp.add("Sd7", Sd7)

    # ---------------- sdf  (enc order [trig36; lin3], ct_enc at base 64)
    s2x, c2x = _trig_perm(3, 6, 0)
    strig_rows = s2x + c2x                    # 36
    wp.add("s_W0_trig", Ws[0][strig_rows, :])
    W0lin4 = np.zeros((4, 256), np.float32)
    W0lin4[0:3] = Ws[0][0:3]
    W0lin4[3] = bs[0]
    wp.add("s_W0_lin4", W0lin4)
    for l in (1, 2, 3, 4, 6, 7):
        wp.add(f"s_W{l}_k0", Ws[l][0:128])
        wp.add(f"s_W{l}_k1", Ws[l][128:256])
    S5 = Ws[5] / SQRT2
    wp.add("s_W5_k0", S5[0:128])
    wp.add("s_W5_k1", S5[128:256])
    wp.add("s_W5_trig", S5[256:295][strig_rows, :])
    W5lin4 = np.zeros((4, 256), np.float32)
    W5lin4[0:3] = S5[256:295][0:3]
    W5lin4[3] = bs[5]
    wp.add("s_W5_lin4", W5lin4)
    # L8 column splits: [0], [1:129], [129:257]
    wp.add("s_W8_sdf_k0", Ws[8][0:128, 0:1])
    wp.add("s_W8_sdf_k1", Ws[8][128:256, 0:1])
    wp.add("s_W8_g1_k0", Ws[8][0:128, 1:129])
    wp.add("s_W8_g1_k1", Ws[8][128:256, 1:129])
    wp.add("s_W8_g2_k0", Ws[8][0:128, 129:257])
    wp.add("s_W8_g2_k1", Ws[8][128:256, 129:257])
    # backward (negated true weights) -> bf16 pack when BF16_VJP
    vp = bp if BF16_VJP else wp
    for l in (1, 2, 3, 4, 6, 7):
        V = -(Ws[l].T)
        vp.add(f"s_V{l}_k0", V[0:128])
        vp.add(f"s_V{l}_k1", V[128:256])
    V5 = -(Ws[5] / SQRT2).T                   # [256, 295]
    vp.add("s_V5h_k0", V5[0:128, 0:256])
    vp.add("s_V5h_k1", V5[128:256, 0:256])
    enc_cols = strig_rows + [0, 1, 2]
    V5e = V5[:, 256:][:, enc_cols]
    vp.add("s_V5e_k0", V5e[0:128])
    vp.add("s_V5e_k1", V5e[128:256])
    V0 = -(Ws[0].T)[:, enc_cols]
    vp.add("s_V0_k0", V0[0:128])
    vp.add("s_V0_k1", V0[128:256])
    # G trig part at base 64 (AB rows 64..99), lin part at base 0
    G = np.zeros((36, 3), np.float32)
    for r in range(18):
        j, i = r // 6, r % 6
        G[r, j] = 2.0 ** i
        G[18 + r, j] = -(2.0 ** i)
    wp.add("G_lin", np.eye(3, dtype=np.float32))
    wp.add("G64", G, base=64, share="G_lin")
    # arg matrices (rhs = XC4 [x_c;ones] or DC4)
    S2u = np.zeros((4, 36), np.float32)
    S2Tu = np.zeros((4, 36), np.float32)
    for r in range(18):
        j, i = r // 6, r % 6
        S2u[j, r] = S2u[j, r + 18] = (2.0 ** i) * INV2PI
        S2Tu[j, r] = S2Tu[j, r + 18] = (2.0 ** i) * INV2PI
    S2u[3, 18:36] = 0.25
    S2Tu[3, 0:18] = 0.25
    wp.add("S2u", S2u)
    wp.add("S2Tu64", S2Tu)                    # out at base 64 (T2)
    wp.add("ones3", np.ones((3, 1), np.float32))
    wp.add("bcast13", np.ones((1, 3), np.float32))

    # ---------------- color (input chunks: trig3@64, trig4, XC4, DC3, GRAD, geo)
    s3x, c3x = _trig_perm(3, 10, 0)
    trig3_rows = s3x + c3x                    # 60
    s4x, c4x = _trig_perm(3, 4, 66)
    trig4_rows = s4x + c4x                    # 24
    W0 = Wc[0]
    wp.add("c_W0_trig3_64", W0[trig3_rows, :], base=64, share="s_W0_trig")
    wp.add("c_W0_trig4", W0[trig4_rows, :])
    cW0lin4 = np.zeros((4, 256), np.float32)
    cW0lin4[0:3] = W0[0:3]                    # x_c lin rows
    cW0lin4[3] = bc[0]                        # bias fold
    wp.add("c_W0_xc4", cW0lin4)
    wp.add("c_W0_dc", W0[66:69, :])
    wp.add("c_W0_grad", W0[63:66, :])
    wp.add("c_W0_geo_k0", W0[93:221, :])
    wp.add("c_W0_geo_k1", W0[221:349, :])
    cwp = bp if BF16_COLOR else wp
    for l in (1, 2, 3):
        cwp.add(f"c_W{l}_k0", Wc[l][0:128])
        cwp.add(f"c_W{l}_k1", Wc[l][128:256])
    cwp.add("c_W4_k0", Wc[4][0:128])           # [128, 3]
    cwp.add("c_W4_k1", Wc[4][128:256])
    S3u = np.zeros((4, 60), np.float32)
    for r in range(30):
        j, i = r // 10, r % 10
        S3u[j, r] = S3u[j, r + 30] = (2.0 ** i) * INV2PI
    S3u[3, 30:60] = 0.25
    wp.add("S3u64", S3u)                      # args3 out at base 64
    S4u = np.zeros((4, 24), np.float32)
    for r in range(12):
        j, i = r // 4, r % 4
        S4u[j, r] = S4u[j, r + 12] = (2.0 ** i) * INV2PI
    S4u[3, 12:24] = 0.25
    wp.add("S4u", S4u)

    # ---------------- consts pack (per-partition bias vectors, [*, 1])
    for l in range(1, 8):
        cp.add(f"d_b{l}_c0", bd[l][0:128, None])
        cp.add(f"d_b{l}_c1", bd[l][128:256, None])
    cp.add("d_b8", bd[8][:, None])            # [3,1]
    for l in (1, 2, 3, 4, 6, 7):
        cp.add(f"s_b{l}_c0", bs[l][0:128, None])
        cp.add(f"s_b{l}_c1", bs[l][128:256, None])
        cp.add(f"s_b{l}100_c0", 100.0 * bs[l][0:128, None])
        cp.add(f"s_b{l}100_c1", 100.0 * bs[l][128:256, None])
    cp.add("s_b8_sdf", bs[8][0:1, None])
    cp.add("s_b8_g1", bs[8][1:129, None])
    cp.add("s_b8_g2", bs[8][129:257, None])
    cp.add("s_w8col0_c0", Ws[8][0:128, 0:1])
    cp.add("s_w8col0_c1", Ws[8][128:256, 0:1])
    for l in (1, 2, 3):
        cp.add(f"c_b{l}_c0", bc[l][0:128, None])
        cp.add(f"c_b{l}_c1", bc[l][128:256, None])
    cp.add("c_b4_half", 0.5 * bc[4][:, None])  # tanh bias
    return wp, cp, bp


# ===================== device program ======================================

_CACHE = {}


def build_program(wp_entries, cp_entries, bp_entries, wcols, ccols, bcols, nt=None):
    """Build the Bass program. *_entries: name -> (off, base, K, M)."""
    if nt is None:
        nt = NT
    nc = bacc.Bacc("TRN2", target_bir_lowering=False, debug=False,
                   num_devices=NCORES)

    inp_d = nc.dram_tensor("inputs", [nt * P, 7], F32, kind="ExternalInput")
    wpack_d = nc.dram_tensor("wpack", [128, wcols], F32, kind="ExternalInput")
    cpack_d = nc.dram_tensor("cpack", [128, ccols], F32, kind="ExternalInput")
    bpack_d = nc.dram_tensor("bpack", [128, max(bcols, 1)], BF16, kind="ExternalInput")
    invs_d = nc.dram_tensor("invs", [1, P], F32, kind="ExternalInput")
    out_d = nc.dram_tensor("out", [nt * P, 8], F32, kind="ExternalOutput")

    act_prev = [None]

    def act(*args, **kwargs):
        ins = nc.scalar.activation(*args, **kwargs).ins
        if ACT_CHAIN and act_prev[0] is not None:
            tile.add_dep_helper(ins, act_prev[0], reason="act table order")
        act_prev[0] = ins
        return ins

    with tile.TileContext(nc) as tc, ExitStack() as ctx:
        konst = ctx.enter_context(tc.tile_pool(name="konst", bufs=1))
        wt = konst.tile([128, wcols], F32, name="wt")
        ct = konst.tile([128, ccols], F32, name="ct")
        invs = konst.tile([1, P], F32, name="invs")
        bt = konst.tile([128, max(bcols, 1)], BF16, name="bt")
        nc.sync.dma_start(out=wt, in_=wpack_d.ap())
        nc.sync.dma_start(out=ct, in_=cpack_d.ap())
        nc.sync.dma_start(out=invs, in_=invs_d.ap())
        if bcols:
            nc.sync.dma_start(out=bt, in_=bpack_d.ap())

        def W(name, msl=None):
            off, base, K, M = wp_entries[name]
            if msl is None:
                return wt[base:base + K, off:off + M]
            return wt[base:base + K, off + msl[0]:off + msl[1]]

        def W2(name, msl=None):
            off, base, K, M = bp_entries[name]
            if msl is None:
                return bt[base:base + K, off:off + M]
            return bt[base:base + K, off + msl[0]:off + msl[1]]

        WV = W2 if BF16_VJP else W
        WC = W2 if BF16_COLOR else W

        def C(name):
            off, base, K, M = cp_entries[name]
            assert M == 1
            return ct[base:base + K, off:off + 1]

        saves = ctx.enter_context(tc.tile_pool(name="saves", bufs=1))
        hsave = saves.tile([128, 16 * P], F32, name="hsave")  # sdf h' 8 layers x 2 chunks

        def hs(l, c):
            return hsave[:, (l * 2 + c) * P:(l * 2 + c + 1) * P]

        work = ctx.enter_context(tc.tile_pool(name="work", bufs=2))
        pwork = ctx.enter_context(tc.tile_pool(name="pwork", bufs=1, space="PSUM"))

        for t in range(nt):
            sl = slice(t * P, (t + 1) * P)

            # ---------------- inputs
            xdt = work.tile([8, P], F32, name="xdt", tag="xdt", bufs=2)
            nc.vector.memset(xdt, 1.0)
            nc.sync.dma_start(out=xdt[0:7, :], in_=inp_d.ap()[sl, :].rearrange("n f -> f n"))
            ddup = work.tile([3, P], F32, name="ddup", tag="ddup", bufs=1)
            nc.sync.dma_start(out=ddup, in_=inp_d.ap()[sl, 3:6].rearrange("n f -> f n"))

            # ---------------- deform encoding
            A1 = pwork.tile([112, P], F32, name="A1", tag="argps", bufs=ARG_BUFS)
            nc.tensor.matmul(A1[0:48, :], W("S7u"), xdt, start=True, stop=True)
            nc.tensor.matmul(A1[64:112, :], W("S7Tu"), xdt, start=True, stop=True)
            V1 = work.tile([112, P], F32, name="V1", tag="vred", bufs=TRANS_BUFS)
            nc.vector.tensor_scalar(V1, A1, MAGIC, MAGIC, ALU.add, ALU.subtract)
            W1 = work.tile([112, P], F32, name="W1", tag="wred", bufs=TRANS_BUFS)
            nc.vector.scalar_tensor_tensor(W1, V1, -1.0, A1, ALU.mult, ALU.add)
            TRIG1 = work.tile([112, P], F32, name="TRIG1", tag="trig1", bufs=1)
            act(TRIG1, W1, AF.Sin, bias=0.0, scale=TWOPI)
            A2 = pwork.tile([112, P], F32, name="A2", tag="argps", bufs=ARG_BUFS)
            nc.tensor.matmul(A2[64:112, :], W("Sd7"), xdt, start=True, stop=True)
            TDOT = work.tile([112, P], F32, name="TDOT", tag="tdot", bufs=1)
            nc.vector.tensor_tensor(TDOT[64:112, :], TRIG1[64:112, :], A2[64:112, :], ALU.mult)

            # ---------------- deform MLP fwd + JVP
            h = [None, None]
            hd = [None, None]
            for l in range(8):
                ph = [pwork.tile([128, P], F32, name=f"ph{l}_{m}", tag=f"ph{m}", bufs=PH_BUFS)
                      for m in range(2)]
                pd = [pwork.tile([128, P], F32, name=f"pd{l}_{m}", tag=f"{PD_TAG}{m}", bufs=(PH_BUFS if PD_TAG == "ph" else PD_BUFS))
                      for m in range(2)]
                for m in range(2):
                    msl = (m * 128, (m + 1) * 128)
                    if l == 0:
                        nc.tensor.matmul(ph[m], W("d_W0_trig", msl), TRIG1[0:48, :], start=True, stop=False)
                        nc.tensor.matmul(ph[m], W("d_W0_lin8", msl), xdt, start=False, stop=True)
                        nc.tensor.matmul(pd[m], W("d_W0_trig64", msl), TDOT[64:112, :], start=True, stop=False)
                        nc.tensor.matmul(pd[m], W("d_W0_lint8", msl), xdt, start=False, stop=True)
                    elif l == 5:
                        nc.tensor.matmul(ph[m], W("d_W5_k0", msl), h[0], start=True, stop=False)
                        nc.tensor.matmul(ph[m], W("d_W5_k1", msl), h[1], start=False, stop=False)
                        nc.tensor.matmul(ph[m], W("d_W5_trig", msl), TRIG1[0:48, :], start=False, stop=False)
                        nc.tensor.matmul(ph[m], W("d_W5_lin8", msl), xdt, start=False, stop=True)
                        nc.tensor.matmul(pd[m], W("d_W5_k0", msl), hd[0], start=True, stop=False)
                        nc.tensor.matmul(pd[m], W("d_W5_k1", msl), hd[1], start=False, stop=False)
                        nc.tensor.matmul(pd[m], W("d_W5_trig64", msl), TDOT[64:112, :], start=False, stop=False)
                        nc.tensor.matmul(pd[m], W("d_W5_lint8", msl), xdt, start=False, stop=True)
                    else:
                        nc.tensor.matmul(ph[m], W(f"d_W{l}_k0", msl), h[0], start=True, stop=False)
                        nc.tensor.matmul(ph[m], W(f"d_W{l}_k1", msl), h[1], start=False, stop=True)
                        nc.tensor.matmul(pd[m], W(f"d_W{l}_k0", msl), hd[0], start=True, stop=False)
                        nc.tensor.matmul(pd[m], W(f"d_W{l}_k1", msl), hd[1], start=False, stop=True)
                nh = [work.tile([128, P], F32, name=f"h{l}_{m}", tag=f"h{m}", bufs=H_BUFS) for m in range(2)]
                nhd = [work.tile([128, P], F32, name=f"hd{l}_{m}", tag=f"hdt{m}", bufs=H_BUFS) for m in range(2)]
                for m in range(2):
                    if l in (0, 5):
                        nc.vector.tensor_scalar(nh[m], ph[m], 0.0, 0.0, ALU.add, ALU.max)
                    else:
                        nc.vector.tensor_scalar(nh[m], ph[m], C(f"d_b{l}_c{m}"), 0.0, ALU.add, ALU.max)
                    nc.vector.scalar_tensor_tensor(nhd[m], nh[m], 0.0, pd[m], ALU.is_gt, ALU.mult)
                h, hd = nh, nhd

            ph8 = pwork.tile([3, P], F32, name="ph8", tag="smallps", bufs=SMALL_BUFS)
            nc.tensor.matmul(ph8, W("d_W8_k0"), h[0], start=True, stop=False)
            nc.tensor.matmul(ph8, W("d_W8_k1"), h[1], start=False, stop=True)
            pd8 = pwork.tile([3, P], F32, name="pd8", tag="smallps", bufs=SMALL_BUFS)
            nc.tensor.matmul(pd8, W("d_W8_k0"), hd[0], start=True, stop=False)
            nc.tensor.matmul(pd8, W("d_W8_k1"), hd[1], start=False, stop=True)

            XC4 = work.tile([4, P], F32, name="XC4", tag="xc4", bufs=LATE_BUFS)
            nc.vector.memset(XC4, 1.0)
            nc.vector.scalar_tensor_tensor(XC4[0:3, :], ph8, C("d_b8"), xdt[0:3, :], ALU.add, ALU.add)
            DCU = work.tile([3, P], F32, name="DCU", tag="dcu", bufs=1)
            nc.vector.tensor_tensor(DCU, pd8, ddup, ALU.add)

            # ---------------- x_c encodings (sin block 1, set 18)
            X1 = pwork.tile([124, P], F32, name="X1", tag="argps", bufs=ARG_BUFS)
            nc.tensor.matmul(X1[0:36, :], W("S2u"), XC4, start=True, stop=True)
            nc.tensor.matmul(X1[64:124, :], W("S3u64"), XC4, start=True, stop=True)
            V2 = work.tile([124, P], F32, name="V2", tag="vred", bufs=TRANS_BUFS)
            nc.vector.tensor_scalar(V2, X1, MAGIC, MAGIC, ALU.add, ALU.subtract)
            W2 = work.tile([124, P], F32, name="W2", tag="wred", bufs=TRANS_BUFS)
            nc.vector.scalar_tensor_tensor(W2, V2, -1.0, X1, ALU.mult, ALU.add)
            TRIGX = work.tile([124, P], F32, name="TRIGX", tag="trigx", bufs=LATE_BUFS)
            act(TRIGX, W2, AF.Sin, bias=0.0, scale=TWOPI)
            # T2 (cos/sin swapped) at base 64 for the vjp AB product
            X2 = pwork.tile([100, P], F32, name="X2", tag="argps", bufs=ARG_BUFS)
            nc.tensor.matmul(X2[64:100, :], W("S2Tu64"), XC4, start=True, stop=True)
            V3 = work.tile([100, P], F32, name="V3", tag="vred", bufs=TRANS_BUFS)
            nc.vector.tensor_scalar(V3[64:100, :], X2[64:100, :], MAGIC, MAGIC, ALU.add, ALU.subtract)
            W3 = work.tile([100, P], F32, name="W3", tag="wred", bufs=TRANS_BUFS)
            nc.vector.scalar_tensor_tensor(W3[64:100, :], V3[64:100, :], -1.0, X2[64:100, :], ALU.mult, ALU.add)
            T2 = work.tile([100, P], F32, name="T2", tag="t2", bufs=LATE_BUFS)
            act(T2[64:100, :], W3[64:100, :], AF.Sin, bias=0.0, scale=TWOPI)

            # ---------------- sdf fwd (set 6: exp/ln)
            sh = [None, None]
            for l in range(8):
                ph = [pwork.tile([128, P], F32, name=f"sph{l}_{m}", tag=f"ph{m}", bufs=PH_BUFS)
                      for m in range(2)]
                for m in range(2):
                    msl = (m * 128, (m + 1) * 128)
                    if l == 0:
                        nc.tensor.matmul(ph[m], W("s_W0_trig", msl), TRIGX[0:36, :], start=True, stop=False)
                        nc.tensor.matmul(ph[m], W("s_W0_lin4", msl), XC4, start=False, stop=True)
                    elif l == 5:
                        nc.tensor.matmul(ph[m], W("s_W5_k0", msl), sh[0], start=True, stop=False)
                        nc.tensor.matmul(ph[m], W("s_W5_k1", msl), sh[1], start=False, stop=False)
                        nc.tensor.matmul(ph[m], W("s_W5_trig", msl), TRIGX[0:36, :], start=False, stop=False)
                        nc.tensor.matmul(ph[m], W("s_W5_lin4", msl), XC4, start=False, stop=True)
                    else:
                        nc.tensor.matmul(ph[m], W(f"s_W{l}_k0", msl), sh[0], start=True, stop=False)
                        nc.tensor.matmul(ph[m], W(f"s_W{l}_k1", msl), sh[1], start=False, stop=True)
                nsh = [hs(l, m) for m in range(2)]
                for m in range(2):
                    az = work.tile([128, P], F32, name=f"az{l}_{m}", tag=f"az{m}", bufs=1)
                    EE = work.tile([128, P], F32, name=f"E{l}_{m}", tag=f"E{m}", bufs=TRANS_BUFS)
                    LL = work.tile([128, P], F32, name=f"L{l}_{m}", tag=f"LL{m}", bufs=1)
                    rz = work.tile([128, P], F32, name=f"rz{l}_{m}", tag=f"rz{m}", bufs=1)
                    if l in (0, 5):
                        act(az, ph[m], AF.Abs, bias=0.0, scale=100.0)
                        nc.vector.tensor_scalar(rz, ph[m], 0.0, 0.0, ALU.add, ALU.max)
                    else:
                        act(az, ph[m], AF.Abs, bias=C(f"s_b{l}100_c{m}"), scale=100.0)
                        nc.vector.tensor_scalar(rz, ph[m], C(f"s_b{l}_c{m}"), 0.0, ALU.add, ALU.max)
                    act(EE, az, AF.Exp, bias=0.0, scale=-1.0)
                    act(LL, EE, AF.Ln, bias=1.0, scale=1.0)
                    nc.vector.scalar_tensor_tensor(nsh[m], LL, 0.01, rz, ALU.mult, ALU.add)
                sh = nsh

            psdf = pwork.tile([1, P], F32, name="psdf", tag="smallps", bufs=SMALL_BUFS)
            nc.tensor.matmul(psdf, W("s_W8_sdf_k0"), sh[0], start=True, stop=False)
            nc.tensor.matmul(psdf, W("s_W8_sdf_k1"), sh[1], start=False, stop=True)
            SDFV = work.tile([1, P], F32, name="SDFV", tag="sdfv", bufs=1)
            nc.vector.tensor_scalar(SDFV, psdf, C("s_b8_sdf"), 0.0, ALU.add, ALU.add)
            GEO = [work.tile([128, P], F32, name=f"GEO{m}", tag=f"geo{m}", bufs=LATE_BUFS) for m in range(2)]
            for m, nm in ((0, "g1"), (1, "g2")):
                pg = pwork.tile([128, P], F32, name=f"pg{m}", tag=f"ph{m}", bufs=PH_BUFS)
                nc.tensor.matmul(pg, W(f"s_W8_{nm}_k0"), sh[0], start=True, stop=False)
                nc.tensor.matmul(pg, W(f"s_W8_{nm}_k1"), sh[1], start=False, stop=True)
                nc.vector.tensor_scalar(GEO[m], pg, C(f"s_b8_{nm}"), 0.0, ALU.add, ALU.add)

            # ---------------- sdf vjp (set 6: exp)
            ctv = [None, None]
            ctenc = pwork.tile([100, P], F32, name="ctenc", tag="ctenc", bufs=1)
            for l in range(7, -1, -1):
                ctX = [work.tile([128, P], BF16 if BF16_VJP else F32,
                                 name=f"ctX{l}_{m}", tag=f"ctx{m}", bufs=CTX_BUFS)
                       for m in range(2)]
                for m in range(2):
                    EP = work.tile([128, P], F32, name=f"EP{l}_{m}", tag=f"E{m}", bufs=TRANS_BUFS)
                    act(EP, hs(l, m), AF.Exp, bias=0.0, scale=-100.0)
                    if l == 7:
                        nc.vector.tensor_scalar(ctX[m], EP, 1.0, C(f"s_w8col0_c{m}"),
                                                ALU.subtract, ALU.mult)
                    else:
                        nc.vector.scalar_tensor_tensor(ctX[m], EP, 1.0, ctv[m],
                                                       ALU.subtract, ALU.mult)
                if l == 0:
                    nc.tensor.matmul(ctenc[64:100, :], WV("s_V0_k0", (0, 36)), ctX[0], start=False, stop=False)
                    nc.tensor.matmul(ctenc[64:100, :], WV("s_V0_k1", (0, 36)), ctX[1], start=False, stop=True)
                    nc.tensor.matmul(ctenc[0:3, :], WV("s_V0_k0", (36, 39)), ctX[0], start=False, stop=False)
                    nc.tensor.matmul(ctenc[0:3, :], WV("s_V0_k1", (36, 39)), ctX[1], start=False, stop=True)
                    break
                nctp = [pwork.tile([128, P], F32, name=f"ctp{l}_{m}", tag=f"{PD_TAG}{m}", bufs=(PH_BUFS if PD_TAG == "ph" else PD_BUFS))
                        for m in range(2)]
                key = "s_V5h" if l == 5 else f"s_V{l}"
                for m in range(2):
                    msl = (m * 128, (m + 1) * 128)
                    nc.tensor.matmul(nctp[m], WV(f"{key}_k0", msl), ctX[0], start=True, stop=False)
                    nc.tensor.matmul(nctp[m], WV(f"{key}_k1", msl), ctX[1], start=False, stop=True)
                if l == 5:
                    nc.tensor.matmul(ctenc[64:100, :], WV("s_V5e_k0", (0, 36)), ctX[0], start=True, stop=False)
                    nc.tensor.matmul(ctenc[64:100, :], WV("s_V5e_k1", (0, 36)), ctX[1], start=False, stop=False)
                    nc.tensor.matmul(ctenc[0:3, :], WV("s_V5e_k0", (36, 39)), ctX[0], start=True, stop=False)
                    nc.tensor.matmul(ctenc[0:3, :], WV("s_V5e_k1", (36, 39)), ctX[1], start=False, stop=False)
                ctv = nctp

            STACK = work.tile([100, P], F32, name="STACK", tag="stack", bufs=1)
            nc.vector.tensor_tensor(STACK[64:100, :], ctenc[64:100, :], T2[64:100, :], ALU.mult)
            CTLIN = work.tile([3, P], F32, name="CTLIN", tag="ctlin", bufs=1)
            nc.vector.tensor_copy(CTLIN, ctenc[0:3, :])
            pgr = pwork.tile([3, P], F32, name="pgr", tag="smallps", bufs=SMALL_BUFS)
            nc.tensor.matmul(pgr, W("G64"), STACK[64:100, :], start=True, stop=False)
            nc.tensor.matmul(pgr, W("G_lin"), CTLIN, start=False, stop=True)
            GRAD = work.tile([3, P], F32, name="GRAD", tag="grad", bufs=LATE_BUFS)
            nc.vector.tensor_copy(GRAD, pgr)

            # ---------------- normalize d_c (set 6: ln/exp)
            SQ = work.tile([3, P], F32, name="SQ", tag="sq", bufs=1)
            act(SQ, DCU, AF.Square)
            pn2 = pwork.tile([1, P], F32, name="pn2", tag="smallps", bufs=SMALL_BUFS)
            nc.tensor.matmul(pn2, W("ones3"), SQ, start=True, stop=True)
            LN2 = work.tile([1, P], F32, name="LN2", tag="ln2", bufs=1)
            act(LN2, pn2, AF.Ln)
            RIN = work.tile([1, P], F32, name="RIN", tag="rin", bufs=1)
            act(RIN, LN2, AF.Exp, bias=0.0, scale=-0.5)
            pbc = pwork.tile([3, P], F32, name="pbc", tag="smallps", bufs=SMALL_BUFS)
            nc.tensor.matmul(pbc, W("bcast13"), RIN, start=True, stop=True)
            DC4 = work.tile([4, P], F32, name="DC4", tag="dc4", bufs=LATE_BUFS)
            nc.vector.memset(DC4, 1.0)
            nc.vector.tensor_tensor(DC4[0:3, :], pbc, DCU, ALU.mult)

            # ---------------- d_c encoding (sin block 2, set 18)
            A4 = pwork.tile([24, P], F32, name="A4", tag="argps", bufs=ARG_BUFS)
            nc.tensor.matmul(A4, W("S4u"), DC4, start=True, stop=True)
            V4 = work.tile([24, P], F32, name="V4", tag="vred", bufs=TRANS_BUFS)
            nc.vector.tensor_scalar(V4, A4, MAGIC, MAGIC, ALU.add, ALU.subtract)
            W4 = work.tile([24, P], F32, name="W4", tag="wred", bufs=TRANS_BUFS)
            nc.vector.scalar_tensor_tensor(W4, V4, -1.0, A4, ALU.mult, ALU.add)
            TRIG4 = work.tile([24, P], F32, name="TRIG4", tag="trig4", bufs=LATE_BUFS)
            act(TRIG4, W4, AF.Sin, bias=0.0, scale=TWOPI)

            # ---------------- color MLP
            chv = [None, None]
            for l in range(4):
                ph = [pwork.tile([128, P], F32, name=f"cph{l}_{m}", tag=f"ph{m}", bufs=PH_BUFS)
                      for m in range(2)]
                for m in range(2):
                    msl = (m * 128, (m + 1) * 128)
                    if l == 0:
                        nc.tensor.matmul(ph[m], W("c_W0_trig3_64", msl), TRIGX[64:124, :], start=True, stop=False)
                        nc.tensor.matmul(ph[m], W("c_W0_trig4", msl), TRIG4, start=False, stop=False)
                        nc.tensor.matmul(ph[m], W("c_W0_xc4", msl), XC4, start=False, stop=False)
                        nc.tensor.matmul(ph[m], W("c_W0_dc", msl), DC4[0:3, :], start=False, stop=False)
                        nc.tensor.matmul(ph[m], W("c_W0_grad", msl), GRAD, start=False, stop=False)
                        nc.tensor.matmul(ph[m], W("c_W0_geo_k0", msl), GEO[0], start=False, stop=False)
                        nc.tensor.matmul(ph[m], W("c_W0_geo_k1", msl), GEO[1], start=False, stop=True)
                    else:
                        nc.tensor.matmul(ph[m], WC(f"c_W{l}_k0", msl), chv[0], start=True, stop=False)
                        nc.tensor.matmul(ph[m], WC(f"c_W{l}_k1", msl), chv[1], start=False, stop=True)
                nh = [work.tile([128, P], BF16 if BF16_COLOR else F32,
                                name=f"ch{l}_{m}", tag=f"chv{m}", bufs=H_BUFS) for m in range(2)]
                for m in range(2):
                    if l == 0:
                        nc.vector.tensor_scalar(nh[m], ph[m], 0.0, 0.0, ALU.add, ALU.max)
                    else:
                        nc.vector.tensor_scalar(nh[m], ph[m], C(f"c_b{l}_c{m}"), 0.0, ALU.add, ALU.max)
                chv = nh
            pcol = pwork.tile([3, P], F32, name="pcol", tag="smallps", bufs=SMALL_BUFS)
            nc.tensor.matmul(pcol, WC("c_W4_k0"), chv[0], start=True, stop=False)
            nc.tensor.matmul(pcol, WC("c_W4_k1"), chv[1], start=False, stop=True)
            TH = work.tile([3, P], F32, name="TH", tag="th", bufs=1)
            act(TH, pcol, AF.Tanh, bias=C("c_b4_half"), scale=0.5)
            COL = work.tile([3, P], F32, name="COL", tag="col", bufs=1)
            nc.vector.tensor_scalar(COL, TH, 0.5, 0.5, ALU.mult, ALU.add)

            # ---------------- outputs
            nc.sync.dma_start(out=out_d.ap()[sl, 0:3].rearrange("n f -> f n"), in_=COL)
            nc.sync.dma_start(out=out_d.ap()[sl, 3:4].rearrange("n f -> f n"), in_=SDFV)
            nc.sync.dma_start(out=out_d.ap()[sl, 4:5].rearrange("n f -> f n"), in_=invs)
            nc.sync.dma_start(out=out_d.ap()[sl, 5:8].rearrange("n f -> f n"), in_=GRAD)

    nc.finalize()
    return nc


# ===================== cached PJRT runner ==================================

class _Runner:
    """Mirrors bass2jax.run_bass_via_pjrt's multi-core path, but caches the
    jitted executable and device-resident weight arrays across calls."""

    def __init__(self, nc):
        import jax
        from jax.sharding import Mesh, PartitionSpec, NamedSharding
        from jax.experimental.shard_map import shard_map
        from concourse import bass2jax
        bass2jax.install_neuronx_cc_hook()
        self.jax = jax
        part = nc.partition_id_tensor.name if nc.partition_id_tensor else None
        in_names, out_names, out_avals, zero_shapes = [], [], [], []
        for alloc in nc.m.functions[0].allocations:
            if not isinstance(alloc, mybir.MemoryLocationSet):
                continue
            name = alloc.memorylocations[0].name
            if alloc.kind == "ExternalInput":
                if name != part:
                    in_names.append(name)
            elif alloc.kind == "ExternalOutput":
                out_names.append(name)
                shape = tuple(alloc.tensor_shape)
                dtype = mybir.dt.np(alloc.dtype)
                out_avals.append(jax.core.ShapedArray(shape, dtype))
                zero_shapes.append((shape, dtype))
        self.in_names = list(in_names)
        self.out_names = out_names
        self.zero_shapes = zero_shapes
        n_params = len(in_names)
        all_names = in_names + out_names + ([part] if part else [])
        donate = tuple(range(n_params, n_params + len(out_names)))

        def _body(*args):
            ops = list(args)
            if part:
                ops.append(bass2jax.partition_id_tensor())
            outs = bass2jax._bass_exec_p.bind(
                *ops, out_avals=tuple(out_avals), in_names=tuple(all_names),
                out_names=tuple(out_names), lowering_input_output_aliases=(),
                sim_require_finite=True, sim_require_nnan=True, nc=nc)
            return tuple(outs)

        devices = jax.devices()[:NCORES]
        self.mesh = Mesh(np.asarray(devices), ("core",))
        spec = PartitionSpec("core")
        nin = n_params + len(out_names)
        self.sharding = NamedSharding(self.mesh, spec)
        self.fn = jax.jit(
            shard_map(_body, mesh=self.mesh, in_specs=(spec,) * nin,
                      out_specs=(spec,) * len(out_names), check_rep=False),
            donate_argnums=donate, keep_unused=True)
        self._dev_cache = {}

    def put_cached(self, key, np_global):
        ent = self._dev_cache.get(key)
        if ent is None or ent[0] != (np_global.shape, str(np_global.dtype)):
            arr = self.jax.device_put(np_global, self.sharding)
            self._dev_cache[key] = ((np_global.shape, str(np_global.dtype)), arr)
            return arr
        return ent[1]

    def make_base_args(self, per_name_global, cache_keys=()):
        args = []
        for n in self.in_names:
            g = per_name_global[n]
            if n in cache_keys:
                import hashlib
                h = hashlib.sha1(np.ascontiguousarray(g)).hexdigest()
                args.append(self.put_cached((n, h), g))
            else:
                args.append(self.jax.device_put(g, self.sharding))
        return args

    def make_zero_args(self):
        out = []
        for shape, dtype in self.zero_shapes:
            z = np.zeros((NCORES * shape[0],) + tuple(shape[1:]), dtype)
            out.append(self.jax.device_put(z, self.sharding))
        return out

    def run(self, args):
        outs = self.fn(*args)
        return [np.asarray(o) for o in outs]


_RUNNERS = {}


# ===================== entry point =========================================

def kernel(inputs, deform_params, sdf_params, color_params, variance):
    inputs = np.ascontiguousarray(np.asarray(inputs, np.float32))
    deform_params = [(np.asarray(w, np.float32), np.asarray(b, np.float32))
                     for (w, b) in deform_params]
    sdf_params = [(np.asarray(w, np.float32), np.asarray(b, np.float32))
                  for (w, b) in sdf_params]
    color_params = [(np.asarray(w, np.float32), np.asarray(b, np.float32))
                    for (w, b) in color_params]
    variance = np.float32(np.asarray(variance))

    import ml_dtypes
    wp, cp, bp = build_packs(deform_params, sdf_params, color_params)
    wpack = wp.materialize()
    cpack = cp.materialize()
    bpack = bp.materialize(ml_dtypes.bfloat16)
    invs_val = np.exp(np.float32(10.0) * variance).astype(np.float32)
    invs_row = np.full((1, P), invs_val, np.float32)

    key = ("prog", wpack.shape[1], cpack.shape[1], bpack.shape[1])
    if key not in _CACHE:
        _CACHE[key] = build_program(wp.entries, cp.entries, bp.entries,
                                    wp.cols, cp.cols, bp.cols)
    nc = _CACHE[key]

    if key not in _RUNNERS:
        _RUNNERS[key] = _Runner(nc)
    runner = _RUNNERS[key]

    glob = {
        "inputs": inputs,
        "wpack": np.concatenate([wpack] * NCORES, 0),
        "cpack": np.concatenate([cpack] * NCORES, 0),
        "bpack": np.concatenate([bpack] * NCORES, 0),
        "invs": np.concatenate([invs_row] * NCORES, 0),
    }
    args = runner.make_base_args(glob, cache_keys=("wpack", "cpack", "bpack")) \
        + runner.make_zero_args()
    outs = runner.run(args)
    out = outs[runner.out_names.index("out")]
    return np.ascontiguousarray(out.astype(np.float32))


def _bench_exec(np_inputs, iters=10):
    """Device-exec-only timing: reuses cached executable + device args."""
    import time
    kernel(**np_inputs)  # ensure compiled + caches warm
    runner = list(_RUNNERS.values())[0]
    inputs = np.ascontiguousarray(np.asarray(np_inputs["inputs"], np.float32))
    wp, cp, bp = build_packs(np_inputs["deform_params"], np_inputs["sdf_params"],
                             np_inputs["color_params"])
    import ml_dtypes
    wpack = wp.materialize()
    cpack = cp.materialize()
    bpack = bp.materialize(ml_dtypes.bfloat16)
    invs_val = np.exp(np.float32(10.0) * np.float32(np.asarray(np_inputs["variance"])))
    invs_row = np.full((1, P), invs_val, np.float32)
    glob = {
        "inputs": inputs,
        "wpack": np.concatenate([wpack] * NCORES, 0),
        "cpack": np.concatenate([cpack] * NCORES, 0),
        "bpack": np.concatenate([bpack] * NCORES, 0),
        "invs": np.concatenate([invs_row] * NCORES, 0),
    }
    base = runner.make_base_args(glob, cache_keys=("wpack", "cpack", "bpack", "invs", "inputs"))
    times = []
    for _ in range(iters):
        zeros = runner.make_zero_args()
        for z in zeros:
            z.block_until_ready()
        t0 = time.perf_counter()
        outs = runner.fn(*base, *zeros)
        for o in outs:
            o.block_until_ready()
        t1 = time.perf_counter()
        times.append(t1 - t0)
    return times


# revision 17
# speedup vs baseline: 871.6510x; 41.1020x over previous
"""Trainium2 Bass kernel for nn_EndoSurfRenderer (EndoSurf-style renderer).

Contract: kernel(**inputs) takes the FULL inputs from setup_inputs() and
returns the FULL [65536, 8] output. Internally shards points across 8
NeuronCores (data parallel), replicates weights, runs one fused Bass/Tile
kernel per core, and gathers.

Pipeline per point-tile (feature-major on-chip layout [features, points]):
  deform MLP fwd + JVP tangent -> x_c, d_c ; sdf MLP fwd (softplus beta=100
  built from exp+ln) ; sdf VJP (sigmoid = 1-exp(-100*h')) -> gradients ;
  d_c normalize via exp(-0.5*ln(|d|^2)) ; color MLP ; assemble [N, 8].

All sin/cos computed as sin(2*pi*w) after exact range reduction
w = u - round(u), u = arg/(2*pi), round via the 1.5*2^23 magic constant.
All matmuls fp32 (fp32r is broken on this silicon; bf16 insufficient for
the beta=100 softplus sensitivity).
"""

import sys
from contextlib import ExitStack

if "/opt/trn_rl_repo" not in sys.path:
    sys.path.insert(0, "/opt/trn_rl_repo")

import numpy as np

import concourse.bacc as bacc
import concourse.tile as tile
from concourse import mybir
from concourse.bass_utils import run_bass_kernel_spmd
import concourse.hw_specs as _hw_specs

_orig_get_tables = _hw_specs.get_activation_tables


def _patched_get_tables(arch):
    tabs = dict(_orig_get_tables(arch))
    out = {}
    target6 = "natural_log_exp_and_others"
    target18 = "silu_and_others"
    move6 = set()
    move18 = set()
    for name, funcs in tabs.items():
        if name == target6:
            move6 = set(funcs)
        if name == target18:
            move18 = set(funcs)
    for name, funcs in tabs.items():
        if name == target6 or name == target18:
            out[name] = funcs
        else:
            keep = {f for f in funcs if f not in move6 and f not in move18}
            out[name] = keep
    return out


_hw_specs.get_activation_tables = _patched_get_tables
bacc.get_activation_tables = _patched_get_tables

F32 = mybir.dt.float32
BF16 = mybir.dt.bfloat16
AF = mybir.ActivationFunctionType
ALU = mybir.AluOpType

NPTS = 65536
NCORES = 8
NPC = NPTS // NCORES        # 8192 points per core
P = 512                     # points per tile
NT = NPC // P               # tiles per core
MAGIC = float(1.5 * 2 ** 23)
TWOPI = float(2.0 * np.pi)
INV2PI = float(1.0 / (2.0 * np.pi))
SQRT2 = np.float32(np.sqrt(2.0))

# tunables (swept via timeline sim)
PH_BUFS = 2
PD_BUFS = 1
PD_TAG = "ph"            # "ph" shares psum tags between h and hd
ARG_BUFS = 2
H_BUFS = 2
E_BUFS = 2
CTX_BUFS = 2
ACT_CHAIN = False
DEF_RELU_ACT = False
BF16_VJP = True
BF16_COLOR = True
SMALL_BUFS = 1
LATE_BUFS = 2
TRANS_BUFS = 1


# ===================== host-side packing (layout registry) ==================

class Pack:
    """Packs many [K, M] weight blocks into one [128, cols] f32 array.
    Each entry is placed at partition rows [base, base+K) and a column
    range; on-device it is read as an SBUF AP slice of one big tile."""

    def __init__(self):
        self.cols = 0
        self.entries = {}   # name -> (off, base, K, M)
        self.arrays = {}

    def add(self, name, arr, base=0, share=None):
        arr = np.ascontiguousarray(arr, np.float32)
        assert arr.ndim == 2
        K, M = arr.shape
        assert base + K <= 128, (name, base, K)
        if share is not None:
            soff, sbase, sK, sM = self.entries[share]
            assert M <= sM and (base >= sbase + sK or base + K <= sbase), (name, share)
            self.entries[name] = (soff, base, K, M)
        else:
            self.entries[name] = (self.cols, base, K, M)
        self.cols += 0 if share is not None else M
        self.arrays[name] = arr

    def materialize(self, dtype=np.float32):
        w = np.zeros((128, max(self.cols, 1)), dtype)
        for name, (off, base, K, M) in self.entries.items():
            w[base:base + K, off:off + M] = self.arrays[name].astype(dtype)
        return w


def _trig_perm(D, L, base):
    sin_idx = [base + D + j * 2 * L + i for j in range(D) for i in range(L)]
    cos_idx = [base + D + j * 2 * L + L + i for j in range(D) for i in range(L)]
    return sin_idx, cos_idx


def build_packs(deform_params, sdf_params, color_params):
    """Returns (wpack, cpack, bpack) Pack objects with every weight block."""
    wp = Pack()
    cp = Pack()
    bp = Pack()

    Wd = [np.asarray(w, np.float32) for (w, b) in deform_params]
    bd = [np.asarray(b, np.float32) for (w, b) in deform_params]
    Ws = [np.asarray(w, np.float32) for (w, b) in sdf_params]
    bs = [np.asarray(b, np.float32) for (w, b) in sdf_params]
    Wc = [np.asarray(w, np.float32) for (w, b) in color_params]
    bc = [np.asarray(b, np.float32) for (w, b) in color_params]

    # ---------------- deform enc helpers
    sx, cx = _trig_perm(3, 6, 0)
    st_, ct_ = _trig_perm(1, 6, 39)
    dtrig_rows = sx + st_ + cx + ct_          # 48: sin(x0..2,t) then cos(...)

    def split_dW0(W):
        trig = W[dtrig_rows, :]
        lin8 = np.zeros((8, W.shape[1]), np.float32)
        lin8[0:3] = W[0:3]
        lin8[6] = W[39]
        lint8 = np.zeros((8, W.shape[1]), np.float32)
        lint8[3:6] = W[0:3]
        return trig, lin8, lint8

    t0, l0, lt0 = split_dW0(Wd[0])
    l0[7] = bd[0]                             # bias via ones-row
    wp.add("d_W0_trig", t0)
    wp.add("d_W0_trig64", t0, base=64, share="d_W0_trig")
    wp.add("d_W0_lin8", l0)
    wp.add("d_W0_lint8", lt0)
    for l in (1, 2, 3, 4, 6, 7):
        wp.add(f"d_W{l}_k0", Wd[l][0:128])
        wp.add(f"d_W{l}_k1", Wd[l][128:256])
    W5 = Wd[5] / SQRT2
    wp.add("d_W5_k0", W5[0:128])
    wp.add("d_W5_k1", W5[128:256])
    t5, l5, lt5 = split_dW0(W5[256:308])
    l5[7] = bd[5]
    wp.add("d_W5_trig", t5)
    wp.add("d_W5_trig64", t5, base=64, share="d_W5_trig")
    wp.add("d_W5_lin8", l5)
    wp.add("d_W5_lint8", lt5)
    wp.add("d_W8_k0", Wd[8][0:128])           # [128, 3]
    wp.add("d_W8_k1", Wd[8][128:256])

    # S matrices (args in u = arg/2pi space, ones-row carries +0.25 cos shift)
    S7u = np.zeros((8, 48), np.float32)
    S7Tu = np.zeros((8, 48), np.float32)
    Sd7 = np.zeros((8, 48), np.float32)
    for r in range(24):
        j, i = r // 6, r % 6
        row = 6 if j == 3 else j
        S7u[row, r] = S7u[row, r + 24] = (2.0 ** i) * INV2PI
        S7Tu[row, r] = S7Tu[row, r + 24] = (2.0 ** i) * INV2PI
        if j < 3:
            Sd7[3 + j, r] = 2.0 ** i
            Sd7[3 + j, r + 24] = -(2.0 ** i)
    S7u[7, 24:48] = 0.25                      # cos rows shift
    S7Tu[7, 0:24] = 0.25                      # T = [cos; sin]
    wp.add("S7u", S7u)
    wp.add("S7Tu", S7Tu)
    wp.add("Sd7", Sd7)

    # ---------------- sdf  (enc order [trig36; lin3], ct_enc at base 64)
    s2x, c2x = _trig_perm(3, 6, 0)
    strig_rows = s2x + c2x                    # 36
    wp.add("s_W0_trig", Ws[0][strig_rows, :])
    W0lin4 = np.zeros((4, 256), np.float32)
    W0lin4[0:3] = Ws[0][0:3]
    W0lin4[3] = bs[0]
    wp.add("s_W0_lin4", W0lin4)
    for l in (1, 2, 3, 4, 6, 7):
        wp.add(f"s_W{l}_k0", Ws[l][0:128])
        wp.add(f"s_W{l}_k1", Ws[l][128:256])
    S5 = Ws[5] / SQRT2
    wp.add("s_W5_k0", S5[0:128])
    wp.add("s_W5_k1", S5[128:256])
    wp.add("s_W5_trig", S5[256:295][strig_rows, :])
    W5lin4 = np.zeros((4, 256), np.float32)
    W5lin4[0:3] = S5[256:295][0:3]
    W5lin4[3] = bs[5]
    wp.add("s_W5_lin4", W5lin4)
    # L8 column splits: [0], [1:129], [129:257]
    wp.add("s_W8_sdf_k0", Ws[8][0:128, 0:1])
    wp.add("s_W8_sdf_k1", Ws[8][128:256, 0:1])
    wp.add("s_W8_g1_k0", Ws[8][0:128, 1:129])
    wp.add("s_W8_g1_k1", Ws[8][128:256, 1:129])
    wp.add("s_W8_g2_k0", Ws[8][0:128, 129:257])
    wp.add("s_W8_g2_k1", Ws[8][128:256, 129:257])
    # backward (negated true weights) -> bf16 pack when BF16_VJP
    vp = bp if BF16_VJP else wp
    for l in (1, 2, 3, 4, 6, 7):
        V = -(Ws[l].T)
        vp.add(f"s_V{l}_k0", V[0:128])
        vp.add(f"s_V{l}_k1", V[128:256])
    V5 = -(Ws[5] / SQRT2).T                   # [256, 295]
    vp.add("s_V5h_k0", V5[0:128, 0:256])
    vp.add("s_V5h_k1", V5[128:256, 0:256])
    enc_cols = strig_rows + [0, 1, 2]
    V5e = V5[:, 256:][:, enc_cols]
    vp.add("s_V5e_k0", V5e[0:128])
    vp.add("s_V5e_k1", V5e[128:256])
    V0 = -(Ws[0].T)[:, enc_cols]
    vp.add("s_V0_k0", V0[0:128])
    vp.add("s_V0_k1", V0[128:256])
    # G trig part at base 64 (AB rows 64..99), lin part at base 0
    G = np.zeros((36, 3), np.float32)
    for r in range(18):
        j, i = r // 6, r % 6
        G[r, j] = 2.0 ** i
        G[18 + r, j] = -(2.0 ** i)
    wp.add("G_lin", np.eye(3, dtype=np.float32))
    wp.add("G64", G, base=64, share="G_lin")
    # arg matrices (rhs = XC4 [x_c;ones] or DC4)
    S2u = np.zeros((4, 36), np.float32)
    S2Tu = np.zeros((4, 36), np.float32)
    for r in range(18):
        j, i = r // 6, r % 6
        S2u[j, r] = S2u[j, r + 18] = (2.0 ** i) * INV2PI
        S2Tu[j, r] = S2Tu[j, r + 18] = (2.0 ** i) * INV2PI
    S2u[3, 18:36] = 0.25
    S2Tu[3, 0:18] = 0.25
    wp.add("S2u", S2u)
    wp.add("S2Tu64", S2Tu)                    # out at base 64 (T2)
    wp.add("ones3", np.ones((3, 1), np.float32))
    wp.add("bcast13", np.ones((1, 3), np.float32))

    # ---------------- color (input chunks: trig3@64, trig4, XC4, DC3, GRAD, geo)
    s3x, c3x = _trig_perm(3, 10, 0)
    trig3_rows = s3x + c3x                    # 60
    s4x, c4x = _trig_perm(3, 4, 66)
    trig4_rows = s4x + c4x                    # 24
    W0 = Wc[0]
    wp.add("c_W0_trig3_64", W0[trig3_rows, :], base=64, share="s_W0_trig")
    wp.add("c_W0_trig4", W0[trig4_rows, :])
    cW0lin4 = np.zeros((4, 256), np.float32)
    cW0lin4[0:3] = W0[0:3]                    # x_c lin rows
    cW0lin4[3] = bc[0]                        # bias fold
    wp.add("c_W0_xc4", cW0lin4)
    wp.add("c_W0_dc", W0[66:69, :])
    wp.add("c_W0_grad", W0[63:66, :])
    wp.add("c_W0_geo_k0", W0[93:221, :])
    wp.add("c_W0_geo_k1", W0[221:349, :])
    cwp = bp if BF16_COLOR else wp
    for l in (1, 2, 3):
        cwp.add(f"c_W{l}_k0", Wc[l][0:128])
        cwp.add(f"c_W{l}_k1", Wc[l][128:256])
    cwp.add("c_W4_k0", Wc[4][0:128])           # [128, 3]
    cwp.add("c_W4_k1", Wc[4][128:256])
    S3u = np.zeros((4, 60), np.float32)
    for r in range(30):
        j, i = r // 10, r % 10
        S3u[j, r] = S3u[j, r + 30] = (2.0 ** i) * INV2PI
    S3u[3, 30:60] = 0.25
    wp.add("S3u64", S3u)                      # args3 out at base 64
    S4u = np.zeros((4, 24), np.float32)
    for r in range(12):
        j, i = r // 4, r % 4
        S4u[j, r] = S4u[j, r + 12] = (2.0 ** i) * INV2PI
    S4u[3, 12:24] = 0.25
    wp.add("S4u", S4u)

    # ---------------- consts pack (per-partition bias vectors, [*, 1])
    for l in range(1, 8):
        cp.add(f"d_b{l}_c0", bd[l][0:128, None])
        cp.add(f"d_b{l}_c1", bd[l][128:256, None])
    cp.add("d_b8", bd[8][:, None])            # [3,1]
    for l in (1, 2, 3, 4, 6, 7):
        cp.add(f"s_b{l}_c0", bs[l][0:128, None])
        cp.add(f"s_b{l}_c1", bs[l][128:256, None])
        cp.add(f"s_b{l}100_c0", 100.0 * bs[l][0:128, None])
        cp.add(f"s_b{l}100_c1", 100.0 * bs[l][128:256, None])
    cp.add("s_b8_sdf", bs[8][0:1, None])
    cp.add("s_b8_g1", bs[8][1:129, None])
    cp.add("s_b8_g2", bs[8][129:257, None])
    cp.add("s_w8col0_c0", Ws[8][0:128, 0:1])
    cp.add("s_w8col0_c1", Ws[8][128:256, 0:1])
    for l in (1, 2, 3):
        cp.add(f"c_b{l}_c0", bc[l][0:128, None])
        cp.add(f"c_b{l}_c1", bc[l][128:256, None])
    cp.add("c_b4_half", 0.5 * bc[4][:, None])  # tanh bias
    return wp, cp, bp


# ===================== device program ======================================

_CACHE = {}


def build_program(wp_entries, cp_entries, bp_entries, wcols, ccols, bcols, nt=None):
    """Build the Bass program. *_entries: name -> (off, base, K, M)."""
    if nt is None:
        nt = NT
    nc = bacc.Bacc("TRN2", target_bir_lowering=False, debug=False,
                   num_devices=NCORES)

    inp_d = nc.dram_tensor("inputs", [nt * P, 7], F32, kind="ExternalInput")
    wpack_d = nc.dram_tensor("wpack", [128, wcols], F32, kind="ExternalInput")
    cpack_d = nc.dram_tensor("cpack", [128, ccols], F32, kind="ExternalInput")
    bpack_d = nc.dram_tensor("bpack", [128, max(bcols, 1)], BF16, kind="ExternalInput")
    invs_d = nc.dram_tensor("invs", [1, P], F32, kind="ExternalInput")
    out_d = nc.dram_tensor("out", [nt * P, 8], F32, kind="ExternalOutput")

    act_prev = [None]

    def act(*args, **kwargs):
        ins = nc.scalar.activation(*args, **kwargs).ins
        if ACT_CHAIN and act_prev[0] is not None:
            tile.add_dep_helper(ins, act_prev[0], reason="act table order")
        act_prev[0] = ins
        return ins

    with tile.TileContext(nc) as tc, ExitStack() as ctx:
        konst = ctx.enter_context(tc.tile_pool(name="konst", bufs=1))
        wt = konst.tile([128, wcols], F32, name="wt")
        ct = konst.tile([128, ccols], F32, name="ct")
        invs = konst.tile([1, P], F32, name="invs")
        bt = konst.tile([128, max(bcols, 1)], BF16, name="bt")
        nc.sync.dma_start(out=wt, in_=wpack_d.ap())
        nc.sync.dma_start(out=ct, in_=cpack_d.ap())
        nc.sync.dma_start(out=invs, in_=invs_d.ap())
        if bcols:
            nc.sync.dma_start(out=bt, in_=bpack_d.ap())

        def W(name, msl=None):
            off, base, K, M = wp_entries[name]
            if msl is None:
                return wt[base:base + K, off:off + M]
            return wt[base:base + K, off + msl[0]:off + msl[1]]

        def W2(name, msl=None):
            off, base, K, M = bp_entries[name]
            if msl is None:
                return bt[base:base + K, off:off + M]
            return bt[base:base + K, off + msl[0]:off + msl[1]]

        WV = W2 if BF16_VJP else W
        WC = W2 if BF16_COLOR else W

        def C(name):
            off, base, K, M = cp_entries[name]
            assert M == 1
            return ct[base:base + K, off:off + 1]

        saves = ctx.enter_context(tc.tile_pool(name="saves", bufs=1))
        hsave = saves.tile([128, 16 * P], F32, name="hsave")  # sdf h' 8 layers x 2 chunks

        def hs(l, c):
            return hsave[:, (l * 2 + c) * P:(l * 2 + c + 1) * P]

        work = ctx.enter_context(tc.tile_pool(name="work", bufs=2))
        pwork = ctx.enter_context(tc.tile_pool(name="pwork", bufs=1, space="PSUM"))

        for t in range(nt):
            sl = slice(t * P, (t + 1) * P)

            # ---------------- inputs
            xdt = work.tile([8, P], F32, name="xdt", tag="xdt", bufs=2)
            nc.vector.memset(xdt, 1.0)
            nc.sync.dma_start(out=xdt[0:7, :], in_=inp_d.ap()[sl, :].rearrange("n f -> f n"))
            ddup = work.tile([3, P], F32, name="ddup", tag="ddup", bufs=1)
            nc.sync.dma_start(out=ddup, in_=inp_d.ap()[sl, 3:6].rearrange("n f -> f n"))

            # ---------------- deform encoding
            A1 = pwork.tile([112, P], F32, name="A1", tag="argps", bufs=ARG_BUFS)
            nc.tensor.matmul(A1[0:48, :], W("S7u"), xdt, start=True, stop=True)
            nc.tensor.matmul(A1[64:112, :], W("S7Tu"), xdt, start=True, stop=True)
            V1 = work.tile([112, P], F32, name="V1", tag="vred", bufs=TRANS_BUFS)
            nc.vector.tensor_scalar(V1, A1, MAGIC, MAGIC, ALU.add, ALU.subtract)
            W1 = work.tile([112, P], F32, name="W1", tag="wred", bufs=TRANS_BUFS)
            nc.vector.scalar_tensor_tensor(W1, V1, -1.0, A1, ALU.mult, ALU.add)
            TRIG1 = work.tile([112, P], F32, name="TRIG1", tag="trig1", bufs=1)
            act(TRIG1, W1, AF.Sin, bias=0.0, scale=TWOPI)
            A2 = pwork.tile([112, P], F32, name="A2", tag="argps", bufs=ARG_BUFS)
            nc.tensor.matmul(A2[64:112, :], W("Sd7"), xdt, start=True, stop=True)
            TDOT = work.tile([112, P], F32, name="TDOT", tag="tdot", bufs=1)
            nc.vector.tensor_tensor(TDOT[64:112, :], TRIG1[64:112, :], A2[64:112, :], ALU.mult)

            # ---------------- deform MLP fwd + JVP
            h = [None, None]
            hd = [None, None]
            for l in range(8):
                ph = [pwork.tile([128, P], F32, name=f"ph{l}_{m}", tag=f"ph{m}", bufs=PH_BUFS)
                      for m in range(2)]
                pd = [pwork.tile([128, P], F32, name=f"pd{l}_{m}", tag=f"{PD_TAG}{m}", bufs=(PH_BUFS if PD_TAG == "ph" else PD_BUFS))
                      for m in range(2)]
                for m in range(2):
                    msl = (m * 128, (m + 1) * 128)
                    if l == 0:
                        nc.tensor.matmul(ph[m], W("d_W0_trig", msl), TRIG1[0:48, :], start=True, stop=False)
                        nc.tensor.matmul(ph[m], W("d_W0_lin8", msl), xdt, start=False, stop=True)
                        nc.tensor.matmul(pd[m], W("d_W0_trig64", msl), TDOT[64:112, :], start=True, stop=False)
                        nc.tensor.matmul(pd[m], W("d_W0_lint8", msl), xdt, start=False, stop=True)
                    elif l == 5:
                        nc.tensor.matmul(ph[m], W("d_W5_k0", msl), h[0], start=True, stop=False)
                        nc.tensor.matmul(ph[m], W("d_W5_k1", msl), h[1], start=False, stop=False)
                        nc.tensor.matmul(ph[m], W("d_W5_trig", msl), TRIG1[0:48, :], start=False, stop=False)
                        nc.tensor.matmul(ph[m], W("d_W5_lin8", msl), xdt, start=False, stop=True)
                        nc.tensor.matmul(pd[m], W("d_W5_k0", msl), hd[0], start=True, stop=False)
                        nc.tensor.matmul(pd[m], W("d_W5_k1", msl), hd[1], start=False, stop=False)
                        nc.tensor.matmul(pd[m], W("d_W5_trig64", msl), TDOT[64:112, :], start=False, stop=False)
                        nc.tensor.matmul(pd[m], W("d_W5_lint8", msl), xdt, start=False, stop=True)
                    else:
                        nc.tensor.matmul(ph[m], W(f"d_W{l}_k0", msl), h[0], start=True, stop=False)
                        nc.tensor.matmul(ph[m], W(f"d_W{l}_k1", msl), h[1], start=False, stop=True)
                        nc.tensor.matmul(pd[m], W(f"d_W{l}_k0", msl), hd[0], start=True, stop=False)
                        nc.tensor.matmul(pd[m], W(f"d_W{l}_k1", msl), hd[1], start=False, stop=True)
                nh = [work.tile([128, P], F32, name=f"h{l}_{m}", tag=f"h{m}", bufs=H_BUFS) for m in range(2)]
                nhd = [work.tile([128, P], F32, name=f"hd{l}_{m}", tag=f"hdt{m}", bufs=H_BUFS) for m in range(2)]
                for m in range(2):
                    if DEF_RELU_ACT:
                        if l in (0, 5):
                            act(nh[m], ph[m], AF.Relu)
                        else:
                            act(nh[m], ph[m], AF.Relu, bias=C(f"d_b{l}_c{m}"), scale=1.0)
                    else:
                        if l in (0, 5):
                            nc.vector.tensor_scalar(nh[m], ph[m], 0.0, 0.0, ALU.add, ALU.max)
                        else:
                            nc.vector.tensor_scalar(nh[m], ph[m], C(f"d_b{l}_c{m}"), 0.0, ALU.add, ALU.max)
                    nc.vector.scalar_tensor_tensor(nhd[m], nh[m], 0.0, pd[m], ALU.is_gt, ALU.mult)
                h, hd = nh, nhd

            ph8 = pwork.tile([3, P], F32, name="ph8", tag="smallps", bufs=SMALL_BUFS)
            nc.tensor.matmul(ph8, W("d_W8_k0"), h[0], start=True, stop=False)
            nc.tensor.matmul(ph8, W("d_W8_k1"), h[1], start=False, stop=True)
            pd8 = pwork.tile([3, P], F32, name="pd8", tag="smallps", bufs=SMALL_BUFS)
            nc.tensor.matmul(pd8, W("d_W8_k0"), hd[0], start=True, stop=False)
            nc.tensor.matmul(pd8, W("d_W8_k1"), hd[1], start=False, stop=True)

            XC4 = work.tile([4, P], F32, name="XC4", tag="xc4", bufs=LATE_BUFS)
            nc.vector.memset(XC4, 1.0)
            nc.vector.scalar_tensor_tensor(XC4[0:3, :], ph8, C("d_b8"), xdt[0:3, :], ALU.add, ALU.add)
            DCU = work.tile([3, P], F32, name="DCU", tag="dcu", bufs=1)
            nc.vector.tensor_tensor(DCU, pd8, ddup, ALU.add)

            # ---------------- x_c encodings (sin block 1, set 18)
            X1 = pwork.tile([124, P], F32, name="X1", tag="argps", bufs=ARG_BUFS)
            nc.tensor.matmul(X1[0:36, :], W("S2u"), XC4, start=True, stop=True)
            nc.tensor.matmul(X1[64:124, :], W("S3u64"), XC4, start=True, stop=True)
            V2 = work.tile([124, P], F32, name="V2", tag="vred", bufs=TRANS_BUFS)
            nc.vector.tensor_scalar(V2, X1, MAGIC, MAGIC, ALU.add, ALU.subtract)
            W2 = work.tile([124, P], F32, name="W2", tag="wred", bufs=TRANS_BUFS)
            nc.vector.scalar_tensor_tensor(W2, V2, -1.0, X1, ALU.mult, ALU.add)
            TRIGX = work.tile([124, P], F32, name="TRIGX", tag="trigx", bufs=LATE_BUFS)
            act(TRIGX, W2, AF.Sin, bias=0.0, scale=TWOPI)
            # T2 (cos/sin swapped) at base 64 for the vjp AB product
            X2 = pwork.tile([100, P], F32, name="X2", tag="argps", bufs=ARG_BUFS)
            nc.tensor.matmul(X2[64:100, :], W("S2Tu64"), XC4, start=True, stop=True)
            V3 = work.tile([100, P], F32, name="V3", tag="vred", bufs=TRANS_BUFS)
            nc.vector.tensor_scalar(V3[64:100, :], X2[64:100, :], MAGIC, MAGIC, ALU.add, ALU.subtract)
            W3 = work.tile([100, P], F32, name="W3", tag="wred", bufs=TRANS_BUFS)
            nc.vector.scalar_tensor_tensor(W3[64:100, :], V3[64:100, :], -1.0, X2[64:100, :], ALU.mult, ALU.add)
            T2 = work.tile([100, P], F32, name="T2", tag="t2", bufs=LATE_BUFS)
            act(T2[64:100, :], W3[64:100, :], AF.Sin, bias=0.0, scale=TWOPI)

            # ---------------- sdf fwd (set 6: exp/ln)
            sh = [None, None]
            for l in range(8):
                ph = [pwork.tile([128, P], F32, name=f"sph{l}_{m}", tag=f"ph{m}", bufs=PH_BUFS)
                      for m in range(2)]
                for m in range(2):
                    msl = (m * 128, (m + 1) * 128)
                    if l == 0:
                        nc.tensor.matmul(ph[m], W("s_W0_trig", msl), TRIGX[0:36, :], start=True, stop=False)
                        nc.tensor.matmul(ph[m], W("s_W0_lin4", msl), XC4, start=False, stop=True)
                    elif l == 5:
                        nc.tensor.matmul(ph[m], W("s_W5_k0", msl), sh[0], start=True, stop=False)
                        nc.tensor.matmul(ph[m], W("s_W5_k1", msl), sh[1], start=False, stop=False)
                        nc.tensor.matmul(ph[m], W("s_W5_trig", msl), TRIGX[0:36, :], start=False, stop=False)
                        nc.tensor.matmul(ph[m], W("s_W5_lin4", msl), XC4, start=False, stop=True)
                    else:
                        nc.tensor.matmul(ph[m], W(f"s_W{l}_k0", msl), sh[0], start=True, stop=False)
                        nc.tensor.matmul(ph[m], W(f"s_W{l}_k1", msl), sh[1], start=False, stop=True)
                nsh = [hs(l, m) for m in range(2)]
                for m in range(2):
                    az = work.tile([128, P], F32, name=f"az{l}_{m}", tag=f"az{m}", bufs=1)
                    EE = work.tile([128, P], F32, name=f"E{l}_{m}", tag=f"E{m}", bufs=TRANS_BUFS)
                    LL = work.tile([128, P], F32, name=f"L{l}_{m}", tag=f"LL{m}", bufs=1)
                    rz = work.tile([128, P], F32, name=f"rz{l}_{m}", tag=f"rz{m}", bufs=1)
                    if l in (0, 5):
                        act(az, ph[m], AF.Abs, bias=0.0, scale=100.0)
                        nc.vector.tensor_scalar(rz, ph[m], 0.0, 0.0, ALU.add, ALU.max)
                    else:
                        act(az, ph[m], AF.Abs, bias=C(f"s_b{l}100_c{m}"), scale=100.0)
                        nc.vector.tensor_scalar(rz, ph[m], C(f"s_b{l}_c{m}"), 0.0, ALU.add, ALU.max)
                    act(EE, az, AF.Exp, bias=0.0, scale=-1.0)
                    act(LL, EE, AF.Ln, bias=1.0, scale=1.0)
                    nc.vector.scalar_tensor_tensor(nsh[m], LL, 0.01, rz, ALU.mult, ALU.add)
                sh = nsh

            psdf = pwork.tile([1, P], F32, name="psdf", tag="smallps", bufs=SMALL_BUFS)
            nc.tensor.matmul(psdf, W("s_W8_sdf_k0"), sh[0], start=True, stop=False)
            nc.tensor.matmul(psdf, W("s_W8_sdf_k1"), sh[1], start=False, stop=True)
            SDFV = work.tile([1, P], F32, name="SDFV", tag="sdfv", bufs=1)
            nc.vector.tensor_scalar(SDFV, psdf, C("s_b8_sdf"), 0.0, ALU.add, ALU.add)
            GEO = [work.tile([128, P], F32, name=f"GEO{m}", tag=f"geo{m}", bufs=LATE_BUFS) for m in range(2)]
            for m, nm in ((0, "g1"), (1, "g2")):
                pg = pwork.tile([128, P], F32, name=f"pg{m}", tag=f"ph{m}", bufs=PH_BUFS)
                nc.tensor.matmul(pg, W(f"s_W8_{nm}_k0"), sh[0], start=True, stop=False)
                nc.tensor.matmul(pg, W(f"s_W8_{nm}_k1"), sh[1], start=False, stop=True)
                nc.vector.tensor_scalar(GEO[m], pg, C(f"s_b8_{nm}"), 0.0, ALU.add, ALU.add)

            # ---------------- sdf vjp (set 6: exp)
            ctv = [None, None]
            ctenc = pwork.tile([100, P], F32, name="ctenc", tag="ctenc", bufs=1)
            for l in range(7, -1, -1):
                ctX = [work.tile([128, P], BF16 if BF16_VJP else F32,
                                 name=f"ctX{l}_{m}", tag=f"ctx{m}", bufs=CTX_BUFS)
                       for m in range(2)]
                for m in range(2):
                    EP = work.tile([128, P], F32, name=f"EP{l}_{m}", tag=f"E{m}", bufs=TRANS_BUFS)
                    act(EP, hs(l, m), AF.Exp, bias=0.0, scale=-100.0)
                    if l == 7:
                        nc.vector.tensor_scalar(ctX[m], EP, 1.0, C(f"s_w8col0_c{m}"),
                                                ALU.subtract, ALU.mult)
                    else:
                        nc.vector.scalar_tensor_tensor(ctX[m], EP, 1.0, ctv[m],
                                                       ALU.subtract, ALU.mult)
                if l == 0:
                    nc.tensor.matmul(ctenc[64:100, :], WV("s_V0_k0", (0, 36)), ctX[0], start=False, stop=False)
                    nc.tensor.matmul(ctenc[64:100, :], WV("s_V0_k1", (0, 36)), ctX[1], start=False, stop=True)
                    nc.tensor.matmul(ctenc[0:3, :], WV("s_V0_k0", (36, 39)), ctX[0], start=False, stop=False)
                    nc.tensor.matmul(ctenc[0:3, :], WV("s_V0_k1", (36, 39)), ctX[1], start=False, stop=True)
                    break
                nctp = [pwork.tile([128, P], F32, name=f"ctp{l}_{m}", tag=f"{PD_TAG}{m}", bufs=(PH_BUFS if PD_TAG == "ph" else PD_BUFS))
                        for m in range(2)]
                key = "s_V5h" if l == 5 else f"s_V{l}"
                for m in range(2):
                    msl = (m * 128, (m + 1) * 128)
                    nc.tensor.matmul(nctp[m], WV(f"{key}_k0", msl), ctX[0], start=True, stop=False)
                    nc.tensor.matmul(nctp[m], WV(f"{key}_k1", msl), ctX[1], start=False, stop=True)
                if l == 5:
                    nc.tensor.matmul(ctenc[64:100, :], WV("s_V5e_k0", (0, 36)), ctX[0], start=True, stop=False)
                    nc.tensor.matmul(ctenc[64:100, :], WV("s_V5e_k1", (0, 36)), ctX[1], start=False, stop=False)
                    nc.tensor.matmul(ctenc[0:3, :], WV("s_V5e_k0", (36, 39)), ctX[0], start=True, stop=False)
                    nc.tensor.matmul(ctenc[0:3, :], WV("s_V5e_k1", (36, 39)), ctX[1], start=False, stop=False)
                ctv = nctp

            STACK = work.tile([100, P], F32, name="STACK", tag="stack", bufs=1)
            nc.vector.tensor_tensor(STACK[64:100, :], ctenc[64:100, :], T2[64:100, :], ALU.mult)
            CTLIN = work.tile([3, P], F32, name="CTLIN", tag="ctlin", bufs=1)
            nc.vector.tensor_copy(CTLIN, ctenc[0:3, :])
            pgr = pwork.tile([3, P], F32, name="pgr", tag="smallps", bufs=SMALL_BUFS)
            nc.tensor.matmul(pgr, W("G64"), STACK[64:100, :], start=True, stop=False)
            nc.tensor.matmul(pgr, W("G_lin"), CTLIN, start=False, stop=True)
            GRAD = work.tile([3, P], F32, name="GRAD", tag="grad", bufs=LATE_BUFS)
            nc.vector.tensor_copy(GRAD, pgr)

            # ---------------- normalize d_c (set 6: ln/exp)
            SQ = work.tile([3, P], F32, name="SQ", tag="sq", bufs=1)
            act(SQ, DCU, AF.Square)
            pn2 = pwork.tile([1, P], F32, name="pn2", tag="smallps", bufs=SMALL_BUFS)
            nc.tensor.matmul(pn2, W("ones3"), SQ, start=True, stop=True)
            LN2 = work.tile([1, P], F32, name="LN2", tag="ln2", bufs=1)
            act(LN2, pn2, AF.Ln)
            RIN = work.tile([1, P], F32, name="RIN", tag="rin", bufs=1)
            act(RIN, LN2, AF.Exp, bias=0.0, scale=-0.5)
            pbc = pwork.tile([3, P], F32, name="pbc", tag="smallps", bufs=SMALL_BUFS)
            nc.tensor.matmul(pbc, W("bcast13"), RIN, start=True, stop=True)
            DC4 = work.tile([4, P], F32, name="DC4", tag="dc4", bufs=LATE_BUFS)
            nc.vector.memset(DC4, 1.0)
            nc.vector.tensor_tensor(DC4[0:3, :], pbc, DCU, ALU.mult)

            # ---------------- d_c encoding (sin block 2, set 18)
            A4 = pwork.tile([24, P], F32, name="A4", tag="argps", bufs=ARG_BUFS)
            nc.tensor.matmul(A4, W("S4u"), DC4, start=True, stop=True)
            V4 = work.tile([24, P], F32, name="V4", tag="vred", bufs=TRANS_BUFS)
            nc.vector.tensor_scalar(V4, A4, MAGIC, MAGIC, ALU.add, ALU.subtract)
            W4 = work.tile([24, P], F32, name="W4", tag="wred", bufs=TRANS_BUFS)
            nc.vector.scalar_tensor_tensor(W4, V4, -1.0, A4, ALU.mult, ALU.add)
            TRIG4 = work.tile([24, P], F32, name="TRIG4", tag="trig4", bufs=LATE_BUFS)
            act(TRIG4, W4, AF.Sin, bias=0.0, scale=TWOPI)

            # ---------------- color MLP
            chv = [None, None]
            for l in range(4):
                ph = [pwork.tile([128, P], F32, name=f"cph{l}_{m}", tag=f"ph{m}", bufs=PH_BUFS)
                      for m in range(2)]
                for m in range(2):
                    msl = (m * 128, (m + 1) * 128)
                    if l == 0:
                        nc.tensor.matmul(ph[m], W("c_W0_trig3_64", msl), TRIGX[64:124, :], start=True, stop=False)
                        nc.tensor.matmul(ph[m], W("c_W0_trig4", msl), TRIG4, start=False, stop=False)
                        nc.tensor.matmul(ph[m], W("c_W0_xc4", msl), XC4, start=False, stop=False)
                        nc.tensor.matmul(ph[m], W("c_W0_dc", msl), DC4[0:3, :], start=False, stop=False)
                        nc.tensor.matmul(ph[m], W("c_W0_grad", msl), GRAD, start=False, stop=False)
                        nc.tensor.matmul(ph[m], W("c_W0_geo_k0", msl), GEO[0], start=False, stop=False)
                        nc.tensor.matmul(ph[m], W("c_W0_geo_k1", msl), GEO[1], start=False, stop=True)
                    else:
                        nc.tensor.matmul(ph[m], WC(f"c_W{l}_k0", msl), chv[0], start=True, stop=False)
                        nc.tensor.matmul(ph[m], WC(f"c_W{l}_k1", msl), chv[1], start=False, stop=True)
                nh = [work.tile([128, P], BF16 if BF16_COLOR else F32,
                                name=f"ch{l}_{m}", tag=f"chv{m}", bufs=H_BUFS) for m in range(2)]
                for m in range(2):
                    if DEF_RELU_ACT:
                        if l == 0:
                            act(nh[m], ph[m], AF.Relu)
                        else:
                            act(nh[m], ph[m], AF.Relu, bias=C(f"c_b{l}_c{m}"), scale=1.0)
                    else:
                        if l == 0:
                            nc.vector.tensor_scalar(nh[m], ph[m], 0.0, 0.0, ALU.add, ALU.max)
                        else:
                            nc.vector.tensor_scalar(nh[m], ph[m], C(f"c_b{l}_c{m}"), 0.0, ALU.add, ALU.max)
                chv = nh
            pcol = pwork.tile([3, P], F32, name="pcol", tag="smallps", bufs=SMALL_BUFS)
            nc.tensor.matmul(pcol, WC("c_W4_k0"), chv[0], start=True, stop=False)
            nc.tensor.matmul(pcol, WC("c_W4_k1"), chv[1], start=False, stop=True)
            TH = work.tile([3, P], F32, name="TH", tag="th", bufs=1)
            act(TH, pcol, AF.Tanh, bias=C("c_b4_half"), scale=0.5)
            COL = work.tile([3, P], F32, name="COL", tag="col", bufs=1)
            nc.vector.tensor_scalar(COL, TH, 0.5, 0.5, ALU.mult, ALU.add)

            # ---------------- outputs
            nc.sync.dma_start(out=out_d.ap()[sl, 0:3].rearrange("n f -> f n"), in_=COL)
            nc.sync.dma_start(out=out_d.ap()[sl, 3:4].rearrange("n f -> f n"), in_=SDFV)
            nc.sync.dma_start(out=out_d.ap()[sl, 4:5].rearrange("n f -> f n"), in_=invs)
            nc.sync.dma_start(out=out_d.ap()[sl, 5:8].rearrange("n f -> f n"), in_=GRAD)

    nc.finalize()
    return nc


# ===================== cached PJRT runner ==================================

class _Runner:
    """Mirrors bass2jax.run_bass_via_pjrt's multi-core path, but caches the
    jitted executable and device-resident weight arrays across calls."""

    def __init__(self, nc):
        import jax
        from jax.sharding import Mesh, PartitionSpec, NamedSharding
        from jax.experimental.shard_map import shard_map
        from concourse import bass2jax
        bass2jax.install_neuronx_cc_hook()
        self.jax = jax
        part = nc.partition_id_tensor.name if nc.partition_id_tensor else None
        in_names, out_names, out_avals, zero_shapes = [], [], [], []
        for alloc in nc.m.functions[0].allocations:
            if not isinstance(alloc, mybir.MemoryLocationSet):
                continue
            name = alloc.memorylocations[0].name
            if alloc.kind == "ExternalInput":
                if name != part:
                    in_names.append(name)
            elif alloc.kind == "ExternalOutput":
                out_names.append(name)
                shape = tuple(alloc.tensor_shape)
                dtype = mybir.dt.np(alloc.dtype)
                out_avals.append(jax.core.ShapedArray(shape, dtype))
                zero_shapes.append((shape, dtype))
        self.in_names = list(in_names)
        self.out_names = out_names
        self.zero_shapes = zero_shapes
        n_params = len(in_names)
        all_names = in_names + out_names + ([part] if part else [])
        donate = tuple(range(n_params, n_params + len(out_names)))

        def _body(*args):
            ops = list(args)
            if part:
                ops.append(bass2jax.partition_id_tensor())
            outs = bass2jax._bass_exec_p.bind(
                *ops, out_avals=tuple(out_avals), in_names=tuple(all_names),
                out_names=tuple(out_names), lowering_input_output_aliases=(),
                sim_require_finite=True, sim_require_nnan=True, nc=nc)
            return tuple(outs)

        devices = jax.devices()[:NCORES]
        self.mesh = Mesh(np.asarray(devices), ("core",))
        spec = PartitionSpec("core")
        nin = n_params + len(out_names)
        self.sharding = NamedSharding(self.mesh, spec)
        self.fn = jax.jit(
            shard_map(_body, mesh=self.mesh, in_specs=(spec,) * nin,
                      out_specs=(spec,) * len(out_names), check_rep=False),
            donate_argnums=donate, keep_unused=True)
        self._dev_cache = {}

    def put_cached(self, key, np_global):
        ent = self._dev_cache.get(key)
        if ent is None or ent[0] != (np_global.shape, str(np_global.dtype)):
            arr = self.jax.device_put(np_global, self.sharding)
            self._dev_cache[key] = ((np_global.shape, str(np_global.dtype)), arr)
            return arr
        return ent[1]

    def make_base_args(self, per_name_global, cache_keys=()):
        args = []
        for n in self.in_names:
            g = per_name_global[n]
            if n in cache_keys:
                import hashlib
                h = hashlib.sha1(np.ascontiguousarray(g)).hexdigest()
                args.append(self.put_cached((n, h), g))
            else:
                args.append(self.jax.device_put(g, self.sharding))
        return args

    def make_zero_args(self):
        out = []
        for shape, dtype in self.zero_shapes:
            z = np.zeros((NCORES * shape[0],) + tuple(shape[1:]), dtype)
            out.append(self.jax.device_put(z, self.sharding))
        return out

    def run(self, args):
        outs = self.fn(*args)
        return [np.asarray(o) for o in outs]


_RUNNERS = {}


# ===================== entry point =========================================

def kernel(inputs, deform_params, sdf_params, color_params, variance):
    inputs = np.ascontiguousarray(np.asarray(inputs, np.float32))
    deform_params = [(np.asarray(w, np.float32), np.asarray(b, np.float32))
                     for (w, b) in deform_params]
    sdf_params = [(np.asarray(w, np.float32), np.asarray(b, np.float32))
                  for (w, b) in sdf_params]
    color_params = [(np.asarray(w, np.float32), np.asarray(b, np.float32))
                    for (w, b) in color_params]
    variance = np.float32(np.asarray(variance))

    import ml_dtypes
    wp, cp, bp = build_packs(deform_params, sdf_params, color_params)
    wpack = wp.materialize()
    cpack = cp.materialize()
    bpack = bp.materialize(ml_dtypes.bfloat16)
    invs_val = np.exp(np.float32(10.0) * variance).astype(np.float32)
    invs_row = np.full((1, P), invs_val, np.float32)

    key = ("prog", wpack.shape[1], cpack.shape[1], bpack.shape[1])
    if key not in _CACHE:
        _CACHE[key] = build_program(wp.entries, cp.entries, bp.entries,
                                    wp.cols, cp.cols, bp.cols)
    nc = _CACHE[key]

    if key not in _RUNNERS:
        _RUNNERS[key] = _Runner(nc)
    runner = _RUNNERS[key]

    glob = {
        "inputs": inputs,
        "wpack": np.concatenate([wpack] * NCORES, 0),
        "cpack": np.concatenate([cpack] * NCORES, 0),
        "bpack": np.concatenate([bpack] * NCORES, 0),
        "invs": np.concatenate([invs_row] * NCORES, 0),
    }
    args = runner.make_base_args(glob, cache_keys=("wpack", "cpack", "bpack")) \
        + runner.make_zero_args()
    outs = runner.run(args)
    out = outs[runner.out_names.index("out")]
    return np.ascontiguousarray(out.astype(np.float32))


def _bench_exec(np_inputs, iters=10):
    """Device-exec-only timing: reuses cached executable + device args."""
    import time
    kernel(**np_inputs)  # ensure compiled + caches warm
    runner = list(_RUNNERS.values())[0]
    inputs = np.ascontiguousarray(np.asarray(np_inputs["inputs"], np.float32))
    wp, cp, bp = build_packs(np_inputs["deform_params"], np_inputs["sdf_params"],
                             np_inputs["color_params"])
    import ml_dtypes
    wpack = wp.materialize()
    cpack = cp.materialize()
    bpack = bp.materialize(ml_dtypes.bfloat16)
    invs_val = np.exp(np.float32(10.0) * np.float32(np.asarray(np_inputs["variance"])))
    invs_row = np.full((1, P), invs_val, np.float32)
    glob = {
        "inputs": inputs,
        "wpack": np.concatenate([wpack] * NCORES, 0),
        "cpack": np.concatenate([cpack] * NCORES, 0),
        "bpack": np.concatenate([bpack] * NCORES, 0),
        "invs": np.concatenate([invs_row] * NCORES, 0),
    }
    base = runner.make_base_args(glob, cache_keys=("wpack", "cpack", "bpack", "invs", "inputs"))
    times = []
    for _ in range(iters):
        zeros = runner.make_zero_args()
        for z in zeros:
            z.block_until_ready()
        t0 = time.perf_counter()
        outs = runner.fn(*base, *zeros)
        for o in outs:
            o.block_until_ready()
        t1 = time.perf_counter()
        times.append(t1 - t0)
    return times
